# revision 13
# baseline (speedup 1.0000x reference)
"""Trainium2 Bass kernel for nn_DecoderCell_59742995087471.

Decoder cell: causal self-attention + add&LN, cross-attention over H + add&LN,
single-Linear FFN + add&LN.  B=2, S=T=2048, D=1024, 16 heads x 64.

Sharding: 8 cores = 2 batch elements x 4 shards.  Within a batch group of 4
cores:
  - queries are assigned STRIDED (core j takes rows j::4 of its batch
    element).  Sorted ascending, the core's 512 queries split into 4
    sub-blocks of 128 whose causal key-range is exactly key tiles
    0..4(g+1)-1 for every core -- so causal skipping is SPMD-uniform and
    attn1 does 62.5% of the full score/PV/exp work.
  - K/V projections are computed sharded: core j projects K/V only for key
    tiles {j, j+4, j+8, j+12} (512 keys) of its batch element, then the
    4-core group AllGathers K^T and V (bf16, ~1MB/rank each) per layer.
    Collectives run on TOPSP/SDMA and overlap compute.

Mask arrives as data only for the 16 diagonal [128k x 128q] tiles and is
applied post-exp on the (otherwise idle) GpSimd/Pool engine.

Layout: activations transposed in SBUF ([d on partitions, rows free]); matmul
operands bf16 (fp32 PSUM accumulate); residual/LN math fp32.  Softmax has no
max-subtraction (scores are O(1) at this data scale) and the denominator
comes from a ones-augmented column in the PV stationary.
"""

import numpy as np
import ml_dtypes

import concourse.bass as bass
import concourse.bacc as bacc
import concourse.mybir as mybir
import concourse.tile as tile

F32 = mybir.dt.float32
BF16 = mybir.dt.bfloat16
AF = mybir.ActivationFunctionType
ALU = mybir.AluOpType

B, S, D, H, HD = 2, 2048, 1024, 16, 64
QL = 512          # query rows per core
NC = 8            # cores
GROUPS = [[0, 1, 2, 3], [4, 5, 6, 7]]
DT = D // 128     # 8 d-tiles
KT = S // 128     # 16 key tiles
PAIRS = H // 2    # 8 head pairs
EPS = 1e-5
VW = H * (HD + 1)  # 1040: interleaved V row width (ones-augmented)

W_NAMES = ["Wq1", "Wk1", "Wv1", "Wo1", "Wq2", "Wk2", "Wv2", "Wo2", "Wf"]
PC = {"bq1": 0, "bk1": 1, "bo1": 2, "g1": 3, "b1": 4,
      "bq2": 5, "bk2": 6, "bo2": 7, "g2": 8, "b2": 9,
      "bf": 10, "g3": 11, "b3": 12}
NPAR = 13

BUFS = {
    "xp": 12,    # [128,512] bf16: xkp -> hkp K/V-piece inputs
    "kT": 3,     # [128,2048] bf16: assembled K^T pair tiles (streamed)
    "v": 18,     # [128,1040] bf16: V tiles (16 resident + prefetch overlap)
    "qT": 9,     # [128,512] bf16 Q^T pairs (8 per layer)
    "sb16": 9,   # [128,512] bf16: x0q, s1_16, s2_16
    "res": 10,   # [128,512] f32 residual stream generations
    "xpre": 2, "xsq": 2,
    "m": 1,      # [128,2048] bf16 diagonal masks (loaded once)
    "p": 4,      # [128,1024] bf16 probs
    "o": 8,      # [128,512] bf16 oT pairs
    "w": 20,     # [128,1024] bf16 weights (rotating)
    "kp": 4,     # [128,512] bf16 K piece outputs awaiting DMA out
    "vp": 2,     # [128,1040] bf16 V piece outputs awaiting DMA out
    "sm": 3,     # [1,512] f32 smalls
    "smb": 2,    # [128,512] f32 broadcasts
    "rb": 2,     # [64,512] f32
    "t1": 2, "t2": 2,  # [128,512] f32 LN temps
}


def _build_body(nc, tc, d, ctx):
    pools = {}

    def _pool(tag, bufs, space="SBUF"):
        if tag not in pools:
            pools[tag] = ctx.enter_context(
                tc.tile_pool(name=tag, bufs=bufs, space=space))
        return pools[tag]

    # create every pool up front (before any instruction is emitted)
    for tag, bufs in BUFS.items():
        _pool(tag, bufs)
    for dt_ in range(DT):
        _pool(f"par{dt_}", 1)
    for tag in ("ones", "eps"):
        _pool(tag, 1)
    for tag, bufs in (("acc", 2), ("pv", 2), ("sc", 2)):
        _pool("ps_" + tag, bufs, space="PSUM")
    dram = ctx.enter_context(tc.tile_pool(name="dram", bufs=1, space="DRAM"))

    def sbt(shape, dtype, tag, name=None):
        return _pool(tag, BUFS[tag]).tile(shape, dtype, tag=tag,
                                          name=name or tag)

    class _PS:
        @staticmethod
        def tile(shape, dtype, tag, bufs, name):
            return _pool("ps_" + tag, bufs, space="PSUM").tile(
                shape, dtype, tag=tag, name=name)
    PS = _PS()

    # ---------------- constants / params ----------------
    par_t = []
    for dt_ in range(DT):
        pt = _pool(f"par{dt_}", 1).tile([128, NPAR], F32, name=f"par{dt_}")
        nc.sync.dma_start(pt, d["par"][dt_ * 128:(dt_ + 1) * 128, :])
        par_t.append(pt)
    ones_t = _pool("ones", 1).tile([128, 1], BF16, name="ones")
    nc.vector.memset(ones_t, 1.0)
    eps_t = _pool("eps", 1).tile([1, 1], F32, name="eps")
    nc.vector.memset(eps_t, EPS)

    def pap(dt_, key):
        c = PC[key]
        return par_t[dt_][:, c:c + 1]

    # ---------------- AG dram buffers ----------------
    k1in = dram.tile([D, QL], BF16, name="k1in", tag="k1in")
    k1out = dram.tile([4 * D, QL], BF16, name="k1out", tag="k1out")
    v1in = dram.tile([QL, VW], BF16, name="v1in", tag="v1in")
    v1out = dram.tile([4 * QL, VW], BF16, name="v1out", tag="v1out")
    k2in = dram.tile([D, QL], BF16, name="k2in", tag="k2in")
    k2out = dram.tile([4 * D, QL], BF16, name="k2out", tag="k2out")
    v2in = dram.tile([QL, VW], BF16, name="v2in", tag="v2in")
    v2out = dram.tile([4 * QL, VW], BF16, name="v2out", tag="v2out")

    # ---------------- building blocks ----------------
    def load_w(name, tag="w"):
        tiles = []
        for dt_ in range(DT):
            t = sbt([128, D], BF16, tag, name=name)
            nc.sync.dma_start(t, d[name][dt_ * 128:(dt_ + 1) * 128, :])
            tiles.append(t)
        return tiles

    def proj_unit(w_t, x_t, out_ap, m, bias_ap, on_act=True):
        """out_ap (bf16 [128,512]) = W[:, m-block].T @ x + bias"""
        acc = PS.tile([128, 512], F32, tag="acc", bufs=2, name="acc")
        for dt_ in range(DT):
            nc.tensor.matmul(acc, w_t[dt_][:, m * 128:(m + 1) * 128],
                             x_t[dt_], start=(dt_ == 0), stop=(dt_ == DT - 1))
        if on_act:
            nc.scalar.activation(out_ap, acc, AF.Identity, bias=bias_ap)
        else:
            nc.vector.tensor_scalar(out_ap, acc, bias_ap, None, op0=ALU.add)
        return acc

    def emit_kv_piece(wk, wv, x_t, bk_key, kin, vin, on_act):
        """Project this core's 512-key shard: K^T piece + interleaved V piece,
        DMA both to the AG input dram tiles."""
        # K^T piece: [1024 dims, 512 keys]
        for m in range(DT):
            t = sbt([128, QL], BF16, "kp", name="kpiece")
            proj_unit(wk, x_t, t, m, pap(m, bk_key), on_act=on_act)
            nc.sync.dma_start(kin[m * 128:(m + 1) * 128, :], t)
        # V piece: per local key tile s, interleaved [128, 1040] + ones col
        for s in range(4):
            vt = sbt([128, VW], BF16, "vp", name="vpiece")
            nc.vector.memset(
                vt.rearrange("p (h c) -> p h c", h=H)[:, :, HD:HD + 1], 1.0)
            for half in range(2):
                acc = PS.tile([128, 512], F32, tag="acc", bufs=2, name="acc")
                for dt_ in range(DT):
                    nc.tensor.matmul(acc, x_t[dt_][:, s * 128:(s + 1) * 128],
                                     wv[dt_][:, half * 512:(half + 1) * 512],
                                     start=(dt_ == 0), stop=(dt_ == DT - 1))
                vv = vt.rearrange("p (h c) -> p h c", h=H)[
                    :, half * 8:(half + 1) * 8, 0:HD]
                av = acc.rearrange("p (h c) -> p h c", h=8)
                nc.vector.tensor_copy(vv, av)
            nc.sync.dma_start(vin[s * 128:(s + 1) * 128, :], vt)

    def emit_ag(inb, outb):
        nc.gpsimd.collective_compute(
            "AllGather", ALU.bypass, replica_groups=GROUPS,
            ins=[inb.opt()], outs=[outb.opt()])

    def emit_q_all(wq, xq_t, bq_key, on_act=True):
        qT = []
        for pair in range(PAIRS):
            qt = sbt([128, QL], BF16, "qT", name="qT")
            proj_unit(wq, xq_t, qt, pair, pap(pair, bq_key), on_act=on_act)
            qT.append(qt)
        return qT

    def load_v_tiles(vout):
        """Load the 16 gathered V tiles.  Global key tile t lives at rank
        block c=t%4, slot s=t//4 -> rows 512*(t%4) + 128*(t//4)."""
        v_ = []
        for t in range(KT):
            vt = sbt([128, VW], BF16, "v", name="vtile")
            r0 = 512 * (t % 4) + 128 * (t // 4)
            nc.sync.dma_start(vt, vout[r0:r0 + 128, :])
            v_.append(vt)
        return v_

    def load_kT_pair(kout, pair):
        """Assemble K^T pair tile [128, 2048] from the AG output.  Rank c's
        block holds key tiles {c, c+4, c+8, c+12} at col slots 0..3."""
        kt_t = sbt([128, S], BF16, "kT", name="kTpair")
        dst = kt_t.rearrange("p (s c k) -> p s c k", s=4, c=4)
        for c in range(4):
            # dst cols for rank c: tiles c, c+4, c+8, c+12 -> [s, c, :] slices
            nc.sync.dma_start(
                dst[:, :, c, :],
                kout[1024 * c + 128 * pair:1024 * c + 128 * (pair + 1), :])
        return kt_t

    def emit_attn(kout, v_t, qT_t, mask_t, causal):
        """Returns 8 oT pair tiles ([128, 512] bf16)."""
        oT_pairs = []
        kT_cur = load_kT_pair(kout, 0)
        for pair in range(PAIRS):
            kT_nxt = load_kT_pair(kout, pair + 1) if pair + 1 < PAIRS else None
            pvs = [PS.tile([HD + 1, QL], F32, tag="pv", bufs=2, name="pv")
                   for _ in range(2)]
            for kt_ in range(KT):
                n = QL - 128 * (kt_ // 4) if causal else QL
                q0 = QL - n
                psc = PS.tile([128, 2 * QL], F32, tag="sc", bufs=2, name="sc")
                for half in range(2):
                    nc.tensor.matmul(
                        psc[:, half * QL:half * QL + n],
                        kT_cur[half * HD:(half + 1) * HD,
                               kt_ * 128:(kt_ + 1) * 128],
                        qT_t[pair][half * HD:(half + 1) * HD, q0:QL],
                        start=True, stop=True)
                pT = sbt([128, 2 * QL], BF16, "p", name="pT")
                pv_in = psc.rearrange("p (h q) -> p h q", h=2)[:, :, 0:n]
                pT_v = pT.rearrange("p (h q) -> p h q", h=2)
                nc.scalar.activation(pT_v[:, :, 0:n], pv_in, AF.Exp,
                                     scale=0.125)
                if causal:
                    # diagonal tile: first 128 active queries of each half
                    mk = mask_t[:, kt_ * 128:(kt_ + 1) * 128]
                    for half in range(2):
                        nc.gpsimd.tensor_mul(
                            pT[:, half * QL:half * QL + 128],
                            pT[:, half * QL:half * QL + 128], mk)
                for half in range(2):
                    h = pair * 2 + half
                    nc.tensor.matmul(
                        pvs[half][:, q0:QL],
                        v_t[kt_][:, h * (HD + 1):h * (HD + 1) + HD + 1],
                        pT[:, half * QL:half * QL + n],
                        start=(kt_ == 0), stop=(kt_ == KT - 1),
                        skip_group_check=True)
            oT = sbt([128, QL], BF16, "o", name="oT")
            for half in range(2):
                recip = sbt([1, QL], F32, "sm", name="recip")
                nc.vector.reciprocal(recip, pvs[half][HD:HD + 1, :])
                rb = sbt([HD, QL], F32, "rb", name="rb")
                nc.gpsimd.partition_broadcast(rb, recip)
                nc.vector.tensor_mul(oT[half * HD:(half + 1) * HD, :],
                                     pvs[half][0:HD, :], rb)
            oT_pairs.append(oT)
            kT_cur = kT_nxt
        return oT_pairs

    def emit_out_proj(w_t, in_pairs, bias_key, resid_t):
        """pre[dt] (f32) = W.T @ in_pairs + bias + resid"""
        pre = []
        for m in range(DT):
            acc = PS.tile([128, 512], F32, tag="acc", bufs=2, name="acc")
            for pr in range(PAIRS):
                nc.tensor.matmul(acc, w_t[pr][:, m * 128:(m + 1) * 128],
                                 in_pairs[pr],
                                 start=(pr == 0), stop=(pr == PAIRS - 1))
            t = sbt([128, QL], F32, "res", name="pre")
            nc.vector.scalar_tensor_tensor(t, acc, pap(m, bias_key), resid_t[m],
                                           op0=ALU.add, op1=ALU.add)
            pre.append(t)
        return pre

    def emit_ln(pre_t, g_key, b_key, want_bf16):
        xb, xq_ = [], []
        for dt_ in range(DT):
            t = sbt([128, QL], BF16, "xpre", name="xpre")
            nc.vector.tensor_copy(t, pre_t[dt_])
            xb.append(t)
            t2_ = sbt([128, QL], BF16, "xsq", name="xsq")
            nc.scalar.square(t2_, pre_t[dt_])
            xq_.append(t2_)
        sx = PS.tile([1, QL], F32, tag="acc", bufs=2, name="acc")
        for dt_ in range(DT):
            nc.tensor.matmul(sx, ones_t, xb[dt_], start=(dt_ == 0),
                             stop=(dt_ == DT - 1), skip_group_check=True)
        sxx = PS.tile([1, QL], F32, tag="acc", bufs=2, name="acc")
        for dt_ in range(DT):
            nc.tensor.matmul(sxx, ones_t, xq_[dt_], start=(dt_ == 0),
                             stop=(dt_ == DT - 1), skip_group_check=True)
        mean = sbt([1, QL], F32, "sm", name="mean")
        nc.vector.tensor_scalar(mean, sx, 1.0 / D, None, op0=ALU.mult)
        meanb = sbt([128, QL], F32, "smb", name="meanb")
        nc.gpsimd.partition_broadcast(meanb, mean)
        msq = sbt([1, QL], F32, "sm", name="msq")
        nc.vector.tensor_mul(msq, mean, mean)
        var = sbt([1, QL], F32, "sm", name="var")
        nc.vector.scalar_tensor_tensor(var, sxx, 1.0 / D, msq,
                                       op0=ALU.mult, op1=ALU.subtract)
        sd = sbt([1, QL], F32, "sm", name="sd")
        nc.scalar.activation(sd, var, AF.Sqrt, bias=eps_t)
        rstd = sbt([1, QL], F32, "sm", name="rstd")
        nc.vector.reciprocal(rstd, sd)
        rstdb = sbt([128, QL], F32, "smb", name="rstdb")
        nc.gpsimd.partition_broadcast(rstdb, rstd)
        out32, out16 = [], []
        for dt_ in range(DT):
            t1 = sbt([128, QL], F32, "t1", name="t1")
            nc.vector.tensor_sub(t1, pre_t[dt_], meanb)
            t2_ = sbt([128, QL], F32, "t2", name="t2")
            nc.vector.tensor_mul(t2_, t1, rstdb)
            o32 = sbt([128, QL], F32, "res", name="lnout")
            nc.vector.tensor_scalar(o32, t2_, pap(dt_, g_key), pap(dt_, b_key),
                                    op0=ALU.mult, op1=ALU.add)
            out32.append(o32)
            if want_bf16:
                o16 = sbt([128, QL], BF16, "sb16", name="lnout16")
                nc.vector.tensor_scalar(o16, t2_, pap(dt_, g_key),
                                        pap(dt_, b_key), op0=ALU.mult,
                                        op1=ALU.add)
                out16.append(o16)
        return out32, out16

    # ---------------- the decoder cell ----------------
    import os
    stop_after = os.environ.get("KSTOP", "")

    def _early_out(tiles):
        for dt_ in range(DT):
            nc.sync.dma_start(d["out"][dt_ * 128:(dt_ + 1) * 128, :], tiles[dt_])
        return True

    # K1/V1 pieces first so AG1 triggers as early as possible
    xkp = []
    for dt_ in range(DT):
        t = sbt([128, QL], BF16, "xp", name="xkp")
        nc.sync.dma_start(t, d["xkp"][dt_ * 128:(dt_ + 1) * 128, :])
        xkp.append(t)
    wk1 = load_w("Wk1")
    wv1 = load_w("Wv1")
    emit_kv_piece(wk1, wv1, xkp, "bk1", k1in, v1in, on_act=True)
    emit_ag(k1in, k1out)
    emit_ag(v1in, v1out)

    # overlap the AG window: Q1, K2/V2 pieces (+ their AGs), bulk loads
    x0q = []
    for dt_ in range(DT):
        t = sbt([128, QL], BF16, "sb16", name="x0q")
        nc.sync.dma_start(t, d["x0q"][dt_ * 128:(dt_ + 1) * 128, :])
        x0q.append(t)
    wq1 = load_w("Wq1")
    q1 = emit_q_all(wq1, x0q, "bq1", on_act=True)

    hkp = []
    for dt_ in range(DT):
        t = sbt([128, QL], BF16, "xp", name="hkp")
        nc.sync.dma_start(t, d["hkp"][dt_ * 128:(dt_ + 1) * 128, :])
        hkp.append(t)
    wk2 = load_w("Wk2")
    wv2 = load_w("Wv2")
    emit_kv_piece(wk2, wv2, hkp, "bk2", k2in, v2in, on_act=True)
    emit_ag(k2in, k2out)
    emit_ag(v2in, v2out)

    msk = sbt([128, S], BF16, "m", name="msk")
    nc.sync.dma_start(msk, d["msk"][:, :])
    x0r = []
    for dt_ in range(DT):
        t = sbt([128, QL], F32, "res", name="x0r")
        nc.sync.dma_start(t, d["x0r"][dt_ * 128:(dt_ + 1) * 128, :])
        x0r.append(t)
    wo1 = load_w("Wo1")
    if stop_after == "qkv1":
        _early_out(x0r); return

    v1 = load_v_tiles(v1out)
    o1 = emit_attn(k1out, v1, q1, msk, causal=True)
    if stop_after == "attn1":
        _early_out(x0r); return

    pre1 = emit_out_proj(wo1, o1, "bo1", x0r)
    s1_32, s1_16 = emit_ln(pre1, "g1", "b1", want_bf16=True)
    if stop_after == "ln1":
        _early_out(s1_32); return

    wq2 = load_w("Wq2")
    q2 = emit_q_all(wq2, s1_16, "bq2", on_act=True)
    v2 = load_v_tiles(v2out)
    o2 = emit_attn(k2out, v2, q2, None, causal=False)
    if stop_after == "attn2":
        _early_out(s1_32); return

    wo2 = load_w("Wo2")
    pre2 = emit_out_proj(wo2, o2, "bo2", s1_32)
    s2_32, s2_16 = emit_ln(pre2, "g2", "b2", want_bf16=True)

    wf = load_w("Wf")
    pre3 = emit_out_proj(wf, s2_16, "bf", s2_32)
    s3_32, _ = emit_ln(pre3, "g3", "b3", want_bf16=False)

    for dt_ in range(DT):
        nc.sync.dma_start(d["out"][dt_ * 128:(dt_ + 1) * 128, :], s3_32[dt_])


_CACHE = {}


def build_program():
    if "nc" in _CACHE:
        return _CACHE["nc"]
    nc = bacc.Bacc("TRN2", target_bir_lowering=False, debug=False,
                   num_devices=NC)
    d = {}
    d["xkp"] = nc.dram_tensor("xkp", [D, QL], BF16, kind="ExternalInput")
    d["hkp"] = nc.dram_tensor("hkp", [D, QL], BF16, kind="ExternalInput")
    d["x0q"] = nc.dram_tensor("x0q", [D, QL], BF16, kind="ExternalInput")
    d["x0r"] = nc.dram_tensor("x0r", [D, QL], F32, kind="ExternalInput")
    d["msk"] = nc.dram_tensor("msk", [128, S], BF16, kind="ExternalInput")
    for w in W_NAMES:
        d[w] = nc.dram_tensor(w, [D, D], BF16, kind="ExternalInput")
    d["par"] = nc.dram_tensor("par", [D, NPAR], F32, kind="ExternalInput")
    d["out"] = nc.dram_tensor("out", [D, QL], F32, kind="ExternalOutput")

    from contextlib import ExitStack
    with tile.TileContext(nc) as tc:
        with ExitStack() as ctx:
            _build_body(nc, tc, {k: (v[:] if hasattr(v, "ap") else v)
                                 for k, v in d.items()}, ctx)
    nc.compile()
    _CACHE["nc"] = nc
    return nc


def _key_cols(j):
    """Global key-column indices of core j's shard: tiles j, j+4, j+8, j+12."""
    return np.concatenate([np.arange(128 * (j + 4 * s), 128 * (j + 4 * s) + 128)
                           for s in range(4)])


def make_in_maps(inputs):
    """Build the 8 per-core input dicts from the full problem inputs."""
    bf = ml_dtypes.bfloat16
    S0 = np.asarray(inputs["S0"], np.float32)
    Hh = np.asarray(inputs["H"], np.float32)

    par = np.zeros((D, NPAR), np.float32)
    for key, col in PC.items():
        src = {"bq1": "bq1", "bk1": "bk1", "bo1": "bo1", "g1": "ln1_g",
               "b1": "ln1_b", "bq2": "bq2", "bk2": "bk2", "bo2": "bo2",
               "g2": "ln2_g", "b2": "ln2_b", "bf": "bf", "g3": "ln3_g",
               "b3": "ln3_b"}[key]
        par[:, col] = np.asarray(inputs[src], np.float32)
    # bv folds exactly into bo: a = (o + bv) @ Wo + bo = o @ Wo + (bv @ Wo + bo)
    par[:, PC["bo1"]] += np.asarray(inputs["bv1"], np.float32) @ np.asarray(
        inputs["Wo1"], np.float32)
    par[:, PC["bo2"]] += np.asarray(inputs["bv2"], np.float32) @ np.asarray(
        inputs["Wo2"], np.float32)

    ws = {w: np.ascontiguousarray(np.asarray(inputs[w], np.float32)).astype(bf)
          for w in W_NAMES}

    in_maps = []
    for c in range(NC):
        b, j = c // 4, c % 4
        kc = _key_cols(j)
        qrows = np.arange(QL) * 4 + j          # strided query rows, ascending
        x0t = np.ascontiguousarray(S0[b].T)
        ht = np.ascontiguousarray(Hh[b].T)
        # diagonal masks: tile kt covers keys [128*kt, 128*kt+128) vs
        # queries q = 4*(128*(kt//4) + i') + j
        msk = np.zeros((128, S), np.float32)
        for kt in range(KT):
            i0 = 128 * (kt // 4)
            q = 4 * (i0 + np.arange(128)) + j
            k = 128 * kt + np.arange(128)
            msk[:, 128 * kt:128 * (kt + 1)] = (k[:, None] <= q[None, :])
        m = {
            "xkp": np.ascontiguousarray(x0t[:, kc]).astype(bf),
            "hkp": np.ascontiguousarray(ht[:, kc]).astype(bf),
            "x0q": np.ascontiguousarray(x0t[:, qrows]).astype(bf),
            "x0r": np.ascontiguousarray(x0t[:, qrows]),
            "msk": msk.astype(bf),
            "par": par,
        }
        m.update(ws)
        in_maps.append(m)
    return in_maps


def kernel(**inputs) -> np.ndarray:
    from concourse.bass_utils import run_bass_kernel_spmd
    nc = build_program()
    in_maps = make_in_maps(inputs)
    res = run_bass_kernel_spmd(nc, in_maps, list(range(NC)))
    _CACHE["last_results"] = res
    out = np.zeros((B, S, D), np.float32)
    for c in range(NC):
        b, j = c // 4, c % 4
        qrows = np.arange(QL) * 4 + j
        out[b, qrows, :] = res.results[c]["out"].T
    return out


# revision 26
# speedup vs baseline: 1.1029x; 1.1029x over previous
"""Trainium2 Bass kernel for nn_DecoderCell_59742995087471.

Decoder cell: causal self-attention + add&LN, cross-attention over H + add&LN,
single-Linear FFN + add&LN.  B=2, S=T=2048, D=1024, 16 heads x 64.

Sharding: 8 cores = 2 batch elements x 4 shards.  Within a batch group of 4
cores:
  - queries are assigned STRIDED (core j takes rows j::4 of its batch
    element).  Sorted ascending, the core's 512 queries split into 4
    sub-blocks of 128 whose causal key-range is exactly key tiles
    0..4(g+1)-1 for every core -- so causal skipping is SPMD-uniform and
    attn1 does 62.5% of the full score/PV/exp work.
  - K/V projections are computed sharded: core j projects K/V only for key
    tiles {j, j+4, j+8, j+12} (512 keys) of its batch element, then the
    4-core group AllGathers K^T and V (bf16, ~1MB/rank each) per layer.
    Collectives run on TOPSP/SDMA and overlap compute.

Mask arrives as data only for the 16 diagonal [128k x 128q] tiles and is
applied post-exp on the (otherwise idle) GpSimd/Pool engine.

Layout: activations transposed in SBUF ([d on partitions, rows free]); matmul
operands bf16 (fp32 PSUM accumulate); residual/LN math fp32.  Softmax has no
max-subtraction (scores are O(1) at this data scale) and the denominator
comes from a ones-augmented column in the PV stationary.
"""

import numpy as np
import ml_dtypes

import concourse.bass as bass
import concourse.bacc as bacc
import concourse.mybir as mybir
import concourse.tile as tile

F32 = mybir.dt.float32
BF16 = mybir.dt.bfloat16
AF = mybir.ActivationFunctionType
ALU = mybir.AluOpType

B, S, D, H, HD = 2, 2048, 1024, 16, 64
QL = 512          # query rows per core
NC = 8            # cores
GROUPS = [[0, 1, 2, 3], [4, 5, 6, 7]]
DT = D // 128     # 8 d-tiles
KT = S // 128     # 16 key tiles
PAIRS = H // 2    # 8 head pairs
EPS = 1e-5
VW = H * (HD + 1)  # 1040: interleaved V row width (ones-augmented)

W_NAMES = ["Wq1", "Wk1", "Wv1", "Wo1", "Wq2", "Wk2", "Wv2", "Wo2", "Wf"]
PC = {"bq1": 0, "bk1": 1, "bo1": 2, "g1": 3, "b1": 4,
      "bq2": 5, "bk2": 6, "bo2": 7, "g2": 8, "b2": 9,
      "bf": 10, "g3": 11, "b3": 12}
NPAR = 13

BUFS = {
    "xp": 12,    # [128,512] bf16: xkp -> hkp K/V-piece inputs
    "kT": 3,     # [128,2048] bf16: assembled K^T pair tiles (streamed)
    "v": 18,     # [128,1040] bf16: V tiles (16 resident + prefetch overlap)
    "qT": 9,     # [128,512] bf16 Q^T pairs (8 per layer)
    "sb16": 9,   # [128,512] bf16: x0q, s1_16, s2_16
    "res": 10,   # [128,512] f32 residual stream generations
    "xpre": 2, "xsq": 2,
    "m": 1,      # [128,2048] bf16 diagonal masks (loaded once)
    "p": 4,      # [128,1024] bf16 probs
    "o": 8,      # [128,512] bf16 oT pairs
    "w": 20,     # [128,1024] bf16 weights (rotating)
    "kp": 4,     # [128,512] bf16 K piece outputs awaiting DMA out
    "vp": 2,     # [128,1040] bf16 V piece outputs awaiting DMA out
    "sm": 4,     # [1,512] smalls
    "rb": 2,     # [64,512] f32 broadcast bounce
    "t1": 2, "t2": 2,  # [128,512] f32 LN temps
}


def _build_body(nc, tc, d, ctx):
    pools = {}

    def _pool(tag, bufs, space="SBUF"):
        if tag not in pools:
            pools[tag] = ctx.enter_context(
                tc.tile_pool(name=tag, bufs=bufs, space=space))
        return pools[tag]

    # create every pool up front (before any instruction is emitted)
    for tag, bufs in BUFS.items():
        _pool(tag, bufs)
    for dt_ in range(DT):
        _pool(f"par{dt_}", 1)
    for tag in ("ones", "eps"):
        _pool(tag, 1)
    for tag, bufs in (("acc", 2), ("pv", 2), ("sc", 2)):
        _pool("ps_" + tag, bufs, space="PSUM")
    dram = ctx.enter_context(tc.tile_pool(name="dram", bufs=1, space="DRAM"))

    def sbt(shape, dtype, tag, name=None):
        return _pool(tag, BUFS[tag]).tile(shape, dtype, tag=tag,
                                          name=name or tag)

    class _PS:
        @staticmethod
        def tile(shape, dtype, tag, bufs, name):
            return _pool("ps_" + tag, bufs, space="PSUM").tile(
                shape, dtype, tag=tag, name=name)
    PS = _PS()

    # ---------------- constants / params ----------------
    par_t = []
    for dt_ in range(DT):
        pt = _pool(f"par{dt_}", 1).tile([128, NPAR], F32, name=f"par{dt_}")
        nc.sync.dma_start(pt, d["par"][dt_ * 128:(dt_ + 1) * 128, :])
        par_t.append(pt)
    # bf16 constants: col 0 = ones column (LN sums); cols 1:129 = 1/D
    # (mean-broadcast stationary); cols 129:193 partition0 = ones row
    # (attn recip row-broadcast stationary)
    onesb = _pool("ones", 1).tile([128, 193], BF16, name="onesb")
    nc.vector.memset(onesb, 1.0)
    nc.vector.memset(onesb[:, 1:129], 1.0 / D)
    ones_t = onesb[:, 0:1]
    meanw_t = onesb[:, 1:129]
    rowb16_t = onesb[0:1, 129:193]
    # f32 constants: [1,128] ones row (rstd broadcast stationary) + eps
    onesf = _pool("eps", 1).tile([1, 129], F32, name="onesf")
    nc.vector.memset(onesf[:, 0:128], 1.0)
    nc.vector.memset(onesf[:, 128:129], EPS)
    rowb32_t = onesf[0:1, 0:128]
    eps_t = onesf[0:1, 128:129]

    def pap(dt_, key):
        c = PC[key]
        return par_t[dt_][:, c:c + 1]

    # ---------------- AG dram buffers ----------------
    k1in = dram.tile([D, QL], BF16, name="k1in", tag="k1in")
    k1out = dram.tile([4 * D, QL], BF16, name="k1out", tag="k1out")
    v1in = dram.tile([QL, VW], BF16, name="v1in", tag="v1in")
    v1out = dram.tile([4 * QL, VW], BF16, name="v1out", tag="v1out")
    k2in = dram.tile([D, QL], BF16, name="k2in", tag="k2in")
    k2out = dram.tile([4 * D, QL], BF16, name="k2out", tag="k2out")
    v2in = dram.tile([QL, VW], BF16, name="v2in", tag="v2in")
    v2out = dram.tile([4 * QL, VW], BF16, name="v2out", tag="v2out")

    # ---------------- building blocks ----------------
    def load_w(name, tag="w"):
        tiles = []
        for dt_ in range(DT):
            t = sbt([128, D], BF16, tag, name=name)
            nc.sync.dma_start(t, d[name][dt_ * 128:(dt_ + 1) * 128, :])
            tiles.append(t)
        return tiles

    def proj_unit(w_t, x_t, out_ap, m, bias_ap, on_act=True):
        """out_ap (bf16 [128,512]) = W[:, m-block].T @ x + bias"""
        acc = PS.tile([128, 512], F32, tag="acc", bufs=2, name="acc")
        for dt_ in range(DT):
            nc.tensor.matmul(acc, w_t[dt_][:, m * 128:(m + 1) * 128],
                             x_t[dt_], start=(dt_ == 0), stop=(dt_ == DT - 1))
        if on_act:
            nc.scalar.activation(out_ap, acc, AF.Identity, bias=bias_ap)
        else:
            nc.vector.tensor_scalar(out_ap, acc, bias_ap, None, op0=ALU.add)
        return acc

    def emit_kv_piece(wk, wv, x_t, bk_key, kin, vin, on_act):
        """Project this core's 512-key shard: K^T piece + interleaved V piece,
        DMA both to the AG input dram tiles."""
        # K^T piece: [1024 dims, 512 keys]
        for m in range(DT):
            t = sbt([128, QL], BF16, "kp", name="kpiece")
            proj_unit(wk, x_t, t, m, pap(m, bk_key), on_act=on_act)
            nc.sync.dma_start(kin[m * 128:(m + 1) * 128, :], t)
        # V piece: per local key tile s, interleaved [128, 1040] + ones col
        for s in range(4):
            vt = sbt([128, VW], BF16, "vp", name="vpiece")
            nc.vector.memset(
                vt.rearrange("p (h c) -> p h c", h=H)[:, :, HD:HD + 1], 1.0)
            for half in range(2):
                acc = PS.tile([128, 512], F32, tag="acc", bufs=2, name="acc")
                for dt_ in range(DT):
                    nc.tensor.matmul(acc, x_t[dt_][:, s * 128:(s + 1) * 128],
                                     wv[dt_][:, half * 512:(half + 1) * 512],
                                     start=(dt_ == 0), stop=(dt_ == DT - 1))
                vv = vt.rearrange("p (h c) -> p h c", h=H)[
                    :, half * 8:(half + 1) * 8, 0:HD]
                av = acc.rearrange("p (h c) -> p h c", h=8)
                nc.vector.tensor_copy(vv, av)
            nc.sync.dma_start(vin[s * 128:(s + 1) * 128, :], vt)

    def emit_ag(inb, outb):
        nc.gpsimd.collective_compute(
            "AllGather", ALU.bypass, replica_groups=GROUPS,
            ins=[inb.opt()], outs=[outb.opt()])

    def emit_q_all(wq, xq_t, bq_key, on_act=True):
        qT = []
        for pair in range(PAIRS):
            qt = sbt([128, QL], BF16, "qT", name="qT")
            proj_unit(wq, xq_t, qt, pair, pap(pair, bq_key), on_act=on_act)
            qT.append(qt)
        return qT

    def load_v_tiles(vout):
        """Load the 16 gathered V tiles.  Global key tile t lives at rank
        block c=t%4, slot s=t//4 -> rows 512*(t%4) + 128*(t//4)."""
        v_ = []
        for t in range(KT):
            vt = sbt([128, VW], BF16, "v", name="vtile")
            r0 = 512 * (t % 4) + 128 * (t // 4)
            nc.gpsimd.dma_start(vt, vout[r0:r0 + 128, :])
            v_.append(vt)
        return v_

    def load_kT_pair(kout, pair):
        """Assemble K^T pair tile [128, 2048] from the AG output, rank-major:
        col block 512*c + 128*s holds global key tile t = c + 4*s (rank c's
        slot s).  Each rank's load is one contiguous [128, 512] DMA."""
        kt_t = sbt([128, S], BF16, "kT", name="kTpair")
        for c in range(4):
            nc.gpsimd.dma_start(
                kt_t[:, 512 * c:512 * (c + 1)],
                kout[1024 * c + 128 * pair:1024 * c + 128 * (pair + 1), :])
        return kt_t

    def kt_col(t):
        """Col block of global key tile t in the rank-major kT pair tile."""
        return 512 * (t % 4) + 128 * (t // 4)

    def emit_attn(kout, v_t, qT_t, mask_t, causal):
        """Returns 8 oT pair tiles ([128, 512] bf16)."""
        oT_pairs = []
        kT_cur = load_kT_pair(kout, 0)
        for pair in range(PAIRS):
            kT_nxt = load_kT_pair(kout, pair + 1) if pair + 1 < PAIRS else None
            pvs = [PS.tile([HD + 1, QL], F32, tag="pv", bufs=2, name="pv")
                   for _ in range(2)]
            for kt_ in range(KT):
                n = QL - 128 * (kt_ // 4) if causal else QL
                q0 = QL - n
                psc = PS.tile([128, 2 * QL], F32, tag="sc", bufs=2, name="sc")
                kc = kt_col(kt_)
                for half in range(2):
                    nc.tensor.matmul(
                        psc[:, half * QL:half * QL + n],
                        kT_cur[half * HD:(half + 1) * HD, kc:kc + 128],
                        qT_t[pair][half * HD:(half + 1) * HD, q0:QL],
                        start=True, stop=True)
                pT = sbt([128, 2 * QL], BF16, "p", name="pT")
                pv_in = psc.rearrange("p (h q) -> p h q", h=2)[:, :, 0:n]
                pT_v = pT.rearrange("p (h q) -> p h q", h=2)
                nc.scalar.activation(pT_v[:, :, 0:n], pv_in, AF.Exp,
                                     scale=0.125)
                if causal:
                    # diagonal tile: first 128 active queries of each half
                    mk = mask_t[:, kt_ * 128:(kt_ + 1) * 128]
                    for half in range(2):
                        nc.vector.tensor_mul(
                            pT[:, half * QL:half * QL + 128],
                            pT[:, half * QL:half * QL + 128], mk)
                for half in range(2):
                    h = pair * 2 + half
                    nc.tensor.matmul(
                        pvs[half][:, q0:QL],
                        v_t[kt_][:, h * (HD + 1):h * (HD + 1) + HD + 1],
                        pT[:, half * QL:half * QL + n],
                        start=(kt_ == 0), stop=(kt_ == KT - 1),
                        skip_group_check=True)
            oT = sbt([128, QL], BF16, "o", name="oT")
            for half in range(2):
                recip = sbt([1, QL], F32, "sm", name="recip")
                nc.vector.reciprocal(recip, pvs[half][HD:HD + 1, :])
                r16 = sbt([1, QL], BF16, "sm", name="r16")
                nc.vector.tensor_copy(r16, recip)
                # row-broadcast recip via PE: [1,64].T @ [1,512] -> [64,512]
                rb = PS.tile([HD, QL], F32, tag="acc", bufs=2, name="rb")
                nc.tensor.matmul(rb, rowb16_t, r16, start=True, stop=True)
                rbs = sbt([HD, QL], F32, "rb", name="rbs")
                nc.vector.tensor_copy(rbs, rb)
                nc.vector.tensor_mul(oT[half * HD:(half + 1) * HD, :],
                                     pvs[half][0:HD, :], rbs)
            oT_pairs.append(oT)
            kT_cur = kT_nxt
        return oT_pairs

    def emit_out_proj(w_t, in_pairs, bias_key, resid_t):
        """pre[dt] (f32) = W.T @ in_pairs + bias + resid"""
        pre = []
        for m in range(DT):
            acc = PS.tile([128, 512], F32, tag="acc", bufs=2, name="acc")
            for pr in range(PAIRS):
                nc.tensor.matmul(acc, w_t[pr][:, m * 128:(m + 1) * 128],
                                 in_pairs[pr],
                                 start=(pr == 0), stop=(pr == PAIRS - 1))
            t = sbt([128, QL], F32, "res", name="pre")
            nc.vector.scalar_tensor_tensor(t, acc, pap(m, bias_key), resid_t[m],
                                           op0=ALU.add, op1=ALU.add)
            pre.append(t)
        return pre

    def emit_ln(pre_t, g_key, b_key, want_bf16):
        xb, xq_ = [], []
        for dt_ in range(DT):
            t = sbt([128, QL], BF16, "xpre", name="xpre")
            nc.vector.tensor_copy(t, pre_t[dt_])
            xb.append(t)
            t2_ = sbt([128, QL], BF16, "xsq", name="xsq")
            nc.scalar.square(t2_, pre_t[dt_])
            xq_.append(t2_)
        # mean, directly broadcast: [128,128]-of-1/D.T @ xb -> [128, 512]
        meanb = PS.tile([128, QL], F32, tag="acc", bufs=2, name="meanb")
        for dt_ in range(DT):
            nc.tensor.matmul(meanb, meanw_t, xb[dt_], start=(dt_ == 0),
                             stop=(dt_ == DT - 1), skip_group_check=True)
        sxx = PS.tile([1, QL], F32, tag="acc", bufs=2, name="sxx")
        for dt_ in range(DT):
            nc.tensor.matmul(sxx, ones_t, xq_[dt_], start=(dt_ == 0),
                             stop=(dt_ == DT - 1), skip_group_check=True)
        mean = sbt([1, QL], F32, "sm", name="mean")
        nc.vector.tensor_copy(mean, meanb[0:1, :])
        msq = sbt([1, QL], F32, "sm", name="msq")
        nc.vector.tensor_mul(msq, mean, mean)
        var = sbt([1, QL], F32, "sm", name="var")
        nc.vector.scalar_tensor_tensor(var, sxx, 1.0 / D, msq,
                                       op0=ALU.mult, op1=ALU.subtract)
        sd = sbt([1, QL], F32, "sm", name="sd")
        nc.scalar.activation(sd, var, AF.Sqrt, bias=eps_t)
        rstd = sbt([1, QL], F32, "sm", name="rstd")
        nc.vector.reciprocal(rstd, sd)
        # row-broadcast rstd via f32 PE matmul: [1,128].T @ [1,512]
        rstdb = PS.tile([128, QL], F32, tag="acc", bufs=2, name="rstdb")
        nc.tensor.matmul(rstdb, rowb32_t, rstd, start=True, stop=True)
        out32, out16 = [], []
        for dt_ in range(DT):
            t1 = sbt([128, QL], F32, "t1", name="t1")
            nc.vector.tensor_sub(t1, pre_t[dt_], meanb)
            t2_ = sbt([128, QL], F32, "t2", name="t2")
            nc.vector.tensor_mul(t2_, t1, rstdb)
            o32 = sbt([128, QL], F32, "res", name="lnout")
            nc.vector.tensor_scalar(o32, t2_, pap(dt_, g_key), pap(dt_, b_key),
                                    op0=ALU.mult, op1=ALU.add)
            out32.append(o32)
            if want_bf16:
                o16 = sbt([128, QL], BF16, "sb16", name="lnout16")
                nc.vector.tensor_scalar(o16, t2_, pap(dt_, g_key),
                                        pap(dt_, b_key), op0=ALU.mult,
                                        op1=ALU.add)
                out16.append(o16)
        return out32, out16

    # ---------------- the decoder cell ----------------
    import os
    stop_after = os.environ.get("KSTOP", "")

    def _early_out(tiles):
        for dt_ in range(DT):
            nc.sync.dma_start(d["out"][dt_ * 128:(dt_ + 1) * 128, :], tiles[dt_])
        return True

    # K1/V1 pieces first so AG1 triggers as early as possible
    xkp = []
    for dt_ in range(DT):
        t = sbt([128, QL], BF16, "xp", name="xkp")
        nc.sync.dma_start(t, d["xkp"][dt_ * 128:(dt_ + 1) * 128, :])
        xkp.append(t)
    wk1 = load_w("Wk1")
    wv1 = load_w("Wv1")
    emit_kv_piece(wk1, wv1, xkp, "bk1", k1in, v1in, on_act=True)
    emit_ag(k1in, k1out)
    emit_ag(v1in, v1out)

    # overlap the AG window: Q1, K2/V2 pieces (+ their AGs), bulk loads
    x0q = []
    for dt_ in range(DT):
        t = sbt([128, QL], BF16, "sb16", name="x0q")
        nc.sync.dma_start(t, d["x0q"][dt_ * 128:(dt_ + 1) * 128, :])
        x0q.append(t)
    wq1 = load_w("Wq1")
    q1 = emit_q_all(wq1, x0q, "bq1", on_act=True)

    hkp = []
    for dt_ in range(DT):
        t = sbt([128, QL], BF16, "xp", name="hkp")
        nc.sync.dma_start(t, d["hkp"][dt_ * 128:(dt_ + 1) * 128, :])
        hkp.append(t)
    wk2 = load_w("Wk2")
    wv2 = load_w("Wv2")
    emit_kv_piece(wk2, wv2, hkp, "bk2", k2in, v2in, on_act=True)
    emit_ag(k2in, k2out)
    emit_ag(v2in, v2out)

    msk = sbt([128, S], BF16, "m", name="msk")
    nc.sync.dma_start(msk, d["msk"][:, :])
    x0r = []
    for dt_ in range(DT):
        t = sbt([128, QL], F32, "res", name="x0r")
        nc.sync.dma_start(t, d["x0r"][dt_ * 128:(dt_ + 1) * 128, :])
        x0r.append(t)
    wo1 = load_w("Wo1")
    if stop_after == "qkv1":
        _early_out(x0r); return

    v1 = load_v_tiles(v1out)
    o1 = emit_attn(k1out, v1, q1, msk, causal=True)
    if stop_after == "attn1":
        _early_out(x0r); return

    pre1 = emit_out_proj(wo1, o1, "bo1", x0r)
    s1_32, s1_16 = emit_ln(pre1, "g1", "b1", want_bf16=True)
    if stop_after == "ln1":
        _early_out(s1_32); return

    wq2 = load_w("Wq2")
    q2 = emit_q_all(wq2, s1_16, "bq2", on_act=True)
    v2 = load_v_tiles(v2out)
    o2 = emit_attn(k2out, v2, q2, None, causal=False)
    if stop_after == "attn2":
        _early_out(s1_32); return

    wo2 = load_w("Wo2")
    pre2 = emit_out_proj(wo2, o2, "bo2", s1_32)
    s2_32, s2_16 = emit_ln(pre2, "g2", "b2", want_bf16=True)

    wf = load_w("Wf")
    pre3 = emit_out_proj(wf, s2_16, "bf", s2_32)
    s3_32, _ = emit_ln(pre3, "g3", "b3", want_bf16=False)

    for dt_ in range(DT):
        nc.sync.dma_start(d["out"][dt_ * 128:(dt_ + 1) * 128, :], s3_32[dt_])


_CACHE = {}


def build_program():
    if "nc" in _CACHE:
        return _CACHE["nc"]
    nc = bacc.Bacc("TRN2", target_bir_lowering=False, debug=False,
                   num_devices=NC)
    d = {}
    d["xkp"] = nc.dram_tensor("xkp", [D, QL], BF16, kind="ExternalInput")
    d["hkp"] = nc.dram_tensor("hkp", [D, QL], BF16, kind="ExternalInput")
    d["x0q"] = nc.dram_tensor("x0q", [D, QL], BF16, kind="ExternalInput")
    d["x0r"] = nc.dram_tensor("x0r", [D, QL], F32, kind="ExternalInput")
    d["msk"] = nc.dram_tensor("msk", [128, S], BF16, kind="ExternalInput")
    for w in W_NAMES:
        d[w] = nc.dram_tensor(w, [D, D], BF16, kind="ExternalInput")
    d["par"] = nc.dram_tensor("par", [D, NPAR], F32, kind="ExternalInput")
    d["out"] = nc.dram_tensor("out", [D, QL], F32, kind="ExternalOutput")

    from contextlib import ExitStack
    with tile.TileContext(nc) as tc:
        with ExitStack() as ctx:
            _build_body(nc, tc, {k: (v[:] if hasattr(v, "ap") else v)
                                 for k, v in d.items()}, ctx)
    nc.compile()
    _CACHE["nc"] = nc
    return nc


def _key_cols(j):
    """Global key-column indices of core j's shard: tiles j, j+4, j+8, j+12."""
    return np.concatenate([np.arange(128 * (j + 4 * s), 128 * (j + 4 * s) + 128)
                           for s in range(4)])


def make_in_maps(inputs):
    """Build the 8 per-core input dicts from the full problem inputs."""
    bf = ml_dtypes.bfloat16
    S0 = np.asarray(inputs["S0"], np.float32)
    Hh = np.asarray(inputs["H"], np.float32)

    par = np.zeros((D, NPAR), np.float32)
    for key, col in PC.items():
        src = {"bq1": "bq1", "bk1": "bk1", "bo1": "bo1", "g1": "ln1_g",
               "b1": "ln1_b", "bq2": "bq2", "bk2": "bk2", "bo2": "bo2",
               "g2": "ln2_g", "b2": "ln2_b", "bf": "bf", "g3": "ln3_g",
               "b3": "ln3_b"}[key]
        par[:, col] = np.asarray(inputs[src], np.float32)
    # bv folds exactly into bo: a = (o + bv) @ Wo + bo = o @ Wo + (bv @ Wo + bo)
    par[:, PC["bo1"]] += np.asarray(inputs["bv1"], np.float32) @ np.asarray(
        inputs["Wo1"], np.float32)
    par[:, PC["bo2"]] += np.asarray(inputs["bv2"], np.float32) @ np.asarray(
        inputs["Wo2"], np.float32)

    ws = {w: np.ascontiguousarray(np.asarray(inputs[w], np.float32)).astype(bf)
          for w in W_NAMES}

    in_maps = []
    for c in range(NC):
        b, j = c // 4, c % 4
        kc = _key_cols(j)
        qrows = np.arange(QL) * 4 + j          # strided query rows, ascending
        x0t = np.ascontiguousarray(S0[b].T)
        ht = np.ascontiguousarray(Hh[b].T)
        # diagonal masks: tile kt covers keys [128*kt, 128*kt+128) vs
        # queries q = 4*(128*(kt//4) + i') + j
        msk = np.zeros((128, S), np.float32)
        for kt in range(KT):
            i0 = 128 * (kt // 4)
            q = 4 * (i0 + np.arange(128)) + j
            k = 128 * kt + np.arange(128)
            msk[:, 128 * kt:128 * (kt + 1)] = (k[:, None] <= q[None, :])
        m = {
            "xkp": np.ascontiguousarray(x0t[:, kc]).astype(bf),
            "hkp": np.ascontiguousarray(ht[:, kc]).astype(bf),
            "x0q": np.ascontiguousarray(x0t[:, qrows]).astype(bf),
            "x0r": np.ascontiguousarray(x0t[:, qrows]),
            "msk": msk.astype(bf),
            "par": par,
        }
        m.update(ws)
        in_maps.append(m)
    return in_maps


def kernel(**inputs) -> np.ndarray:
    from concourse.bass_utils import run_bass_kernel_spmd
    nc = build_program()
    in_maps = make_in_maps(inputs)
    res = run_bass_kernel_spmd(nc, in_maps, list(range(NC)))
    _CACHE["last_results"] = res
    out = np.zeros((B, S, D), np.float32)
    for c in range(NC):
        b, j = c // 4, c % 4
        qrows = np.arange(QL) * 4 + j
        out[b, qrows, :] = res.results[c]["out"].T
    return out


# revision 34
# speedup vs baseline: 1.2338x; 1.1187x over previous
"""Trainium2 Bass kernel for nn_DecoderCell_59742995087471.

Decoder cell: causal self-attention + add&LN, cross-attention over H + add&LN,
single-Linear FFN + add&LN.  B=2, S=T=2048, D=1024, 16 heads x 64.

Sharding: 8 cores = 2 batch elements x 4 shards.  Within a batch group of 4
cores:
  - queries are assigned STRIDED (core j takes rows j::4 of its batch
    element).  Sorted ascending, the core's 512 queries split into 4
    sub-blocks of 128 whose causal key-range is exactly key tiles
    0..4(g+1)-1 for every core -- so causal skipping is SPMD-uniform and
    attn1 does 62.5% of the full score/PV/exp work.
  - K/V projections are computed sharded: core j projects K/V only for key
    tiles {j, j+4, j+8, j+12} (512 keys) of its batch element, then the
    4-core group AllGathers K^T and V (bf16, ~1MB/rank each) per layer.
    Collectives run on TOPSP/SDMA and overlap compute.

Mask arrives as data only for the 16 diagonal [128k x 128q] tiles and is
applied post-exp on the (otherwise idle) GpSimd/Pool engine.

Layout: activations transposed in SBUF ([d on partitions, rows free]); matmul
operands bf16 (fp32 PSUM accumulate); residual/LN math fp32.  Softmax has no
max-subtraction (scores are O(1) at this data scale) and the denominator
comes from a ones-augmented column in the PV stationary.
"""

import numpy as np
import ml_dtypes

import concourse.bass as bass
import concourse.bacc as bacc
import concourse.mybir as mybir
import concourse.tile as tile

F32 = mybir.dt.float32
BF16 = mybir.dt.bfloat16
FP8 = mybir.dt.float8e4
AF = mybir.ActivationFunctionType
ALU = mybir.AluOpType

FP8_AG = True          # AllGather K/V in fp8e4m3 (half wire bytes)
KV_DT = FP8 if FP8_AG else BF16

B, S, D, H, HD = 2, 2048, 1024, 16, 64
QL = 512          # query rows per core
NC = 8            # cores
GROUPS = [[0, 1, 2, 3], [4, 5, 6, 7]]
DT = D // 128     # 8 d-tiles
KT = S // 128     # 16 key tiles
PAIRS = H // 2    # 8 head pairs
EPS = 1e-5
VW = H * (HD + 1)  # 1040: interleaved V row width (ones-augmented)

W_NAMES = ["Wq1", "Wk1", "Wv1", "Wo1", "Wq2", "Wk2", "Wv2", "Wo2", "Wf"]
PC = {"bq1": 0, "bk1": 1, "bo1": 2, "g1": 3, "b1": 4,
      "bq2": 5, "bk2": 6, "bo2": 7, "g2": 8, "b2": 9,
      "bf": 10, "g3": 11, "b3": 12}
NPAR = 13

BUFS = {
    "xp": 12,    # [128,512] bf16: xkp -> hkp K/V-piece inputs
    "kT": 9,     # [128,2048] kv-dt: assembled K^T pair tiles (all 8 + slack)
    "v": 18,     # [128,1040] kv-dt: V tiles (16 resident + prefetch overlap)
    "qT": 9,     # [128,512] bf16 Q^T pairs (8 per layer)
    "sb16": 9,   # [128,512] bf16: x0q, s1_16, s2_16
    "res": 10,   # [128,512] f32 residual stream generations
    "xpre": 2, "xsq": 2,
    "m": 1,      # [128,2048] bf16 diagonal masks (loaded once)
    "p": 4,      # [128,1024] bf16 probs
    "o": 8,      # [128,512] bf16 oT pairs
    "w": 20,     # [128,1024] bf16 weights (rotating)
    "kp": 4,     # [128,512] bf16 K piece outputs awaiting DMA out
    "vp": 2,     # [128,1040] bf16 V piece outputs awaiting DMA out
    "sm": 4,     # [1,512] smalls
    "rb": 2,     # [64,512] f32 broadcast bounce
    "t1": 2, "t2": 2,  # [128,512] f32 LN temps
}


def _build_body(nc, tc, d, ctx):
    pools = {}

    def _pool(tag, bufs, space="SBUF"):
        if tag not in pools:
            pools[tag] = ctx.enter_context(
                tc.tile_pool(name=tag, bufs=bufs, space=space))
        return pools[tag]

    # create every pool up front (before any instruction is emitted)
    for tag, bufs in BUFS.items():
        _pool(tag, bufs)
    for dt_ in range(DT):
        _pool(f"par{dt_}", 1)
    for tag in ("ones", "eps"):
        _pool(tag, 1)
    for tag, bufs in (("acc", 2), ("pv", 2), ("sc", 2)):
        _pool("ps_" + tag, bufs, space="PSUM")
    dram = ctx.enter_context(tc.tile_pool(name="dram", bufs=1, space="DRAM"))

    def sbt(shape, dtype, tag, name=None):
        return _pool(tag, BUFS[tag]).tile(shape, dtype, tag=tag,
                                          name=name or tag)

    class _PS:
        @staticmethod
        def tile(shape, dtype, tag, bufs, name):
            return _pool("ps_" + tag, bufs, space="PSUM").tile(
                shape, dtype, tag=tag, name=name)
    PS = _PS()

    # ---------------- constants / params ----------------
    par_t = []
    for dt_ in range(DT):
        pt = _pool(f"par{dt_}", 1).tile([128, NPAR], F32, name=f"par{dt_}")
        nc.sync.dma_start(pt, d["par"][dt_ * 128:(dt_ + 1) * 128, :])
        par_t.append(pt)
    # bf16 constants: col 0 = ones column (LN sums); cols 1:129 = 1/D
    # (mean-broadcast stationary); cols 129:193 partition0 = ones row
    # (attn recip row-broadcast stationary)
    onesb = _pool("ones", 1).tile([128, 193], BF16, name="onesb")
    nc.vector.memset(onesb, 1.0)
    nc.vector.memset(onesb[:, 1:129], 1.0 / D)
    ones_t = onesb[:, 0:1]
    meanw_t = onesb[:, 1:129]
    rowb16_t = onesb[0:1, 129:193]
    # f32 constants: [1,128] ones row (rstd broadcast stationary) + eps
    onesf = _pool("eps", 1).tile([1, 129], F32, name="onesf")
    nc.vector.memset(onesf[:, 0:128], 1.0)
    nc.vector.memset(onesf[:, 128:129], EPS)
    rowb32_t = onesf[0:1, 0:128]
    eps_t = onesf[0:1, 128:129]

    def pap(dt_, key):
        c = PC[key]
        return par_t[dt_][:, c:c + 1]

    # ---------------- AG dram buffers ----------------
    k1in = dram.tile([D, QL], KV_DT, name="k1in", tag="k1in")
    k1out = dram.tile([4 * D, QL], KV_DT, name="k1out", tag="k1out")
    v1in = dram.tile([QL, VW], KV_DT, name="v1in", tag="v1in")
    v1out = dram.tile([4 * QL, VW], KV_DT, name="v1out", tag="v1out")
    k2in = dram.tile([D, QL], KV_DT, name="k2in", tag="k2in")
    k2out = dram.tile([4 * D, QL], KV_DT, name="k2out", tag="k2out")
    v2in = dram.tile([QL, VW], KV_DT, name="v2in", tag="v2in")
    v2out = dram.tile([4 * QL, VW], KV_DT, name="v2out", tag="v2out")

    # ---------------- building blocks ----------------
    def load_w(name, tag="w"):
        tiles = []
        for dt_ in range(DT):
            t = sbt([128, D], BF16, tag, name=name)
            nc.sync.dma_start(t, d[name][dt_ * 128:(dt_ + 1) * 128, :])
            tiles.append(t)
        return tiles

    def proj_unit(w_t, x_t, out_ap, m, bias_ap, on_act=True):
        """out_ap (bf16 [128,512]) = W[:, m-block].T @ x + bias"""
        acc = PS.tile([128, 512], F32, tag="acc", bufs=2, name="acc")
        for dt_ in range(DT):
            nc.tensor.matmul(acc, w_t[dt_][:, m * 128:(m + 1) * 128],
                             x_t[dt_], start=(dt_ == 0), stop=(dt_ == DT - 1))
        if on_act:
            nc.scalar.activation(out_ap, acc, AF.Identity, bias=bias_ap)
        else:
            nc.vector.tensor_scalar(out_ap, acc, bias_ap, None, op0=ALU.add)
        return acc

    def emit_kv_piece(wk, wv, x_t, bk_key, kin, vin, on_act):
        """Project this core's 512-key shard: K^T piece + interleaved V piece,
        DMA both to the AG input dram tiles."""
        # K^T piece: [1024 dims, 512 keys]
        for m in range(DT):
            t = sbt([128, QL], KV_DT, "kp", name="kpiece")
            proj_unit(wk, x_t, t, m, pap(m, bk_key), on_act=on_act)
            nc.sync.dma_start(kin[m * 128:(m + 1) * 128, :], t)
        # V piece: per local key tile s, interleaved [128, 1040] + ones col
        for s in range(4):
            vt = sbt([128, VW], KV_DT, "vp", name="vpiece")
            nc.vector.memset(
                vt.rearrange("p (h c) -> p h c", h=H)[:, :, HD:HD + 1], 1.0)
            for half in range(2):
                acc = PS.tile([128, 512], F32, tag="acc", bufs=2, name="acc")
                for dt_ in range(DT):
                    nc.tensor.matmul(acc, x_t[dt_][:, s * 128:(s + 1) * 128],
                                     wv[dt_][:, half * 512:(half + 1) * 512],
                                     start=(dt_ == 0), stop=(dt_ == DT - 1))
                vv = vt.rearrange("p (h c) -> p h c", h=H)[
                    :, half * 8:(half + 1) * 8, 0:HD]
                av = acc.rearrange("p (h c) -> p h c", h=8)
                nc.vector.tensor_copy(vv, av)
            nc.sync.dma_start(vin[s * 128:(s + 1) * 128, :], vt)

    def emit_ag(inb, outb):
        nc.gpsimd.collective_compute(
            "AllGather", ALU.bypass, replica_groups=GROUPS,
            ins=[inb.opt()], outs=[outb.opt()])

    def emit_q_all(wq, xq_t, bq_key, on_act=True):
        qT = []
        for pair in range(PAIRS):
            qt = sbt([128, QL], BF16, "qT", name="qT")
            proj_unit(wq, xq_t, qt, pair, pap(pair, bq_key), on_act=on_act)
            qT.append(qt)
        return qT

    def load_v_tiles(vout):
        """Load the 16 gathered V tiles.  Global key tile t lives at rank
        block c=t%4, slot s=t//4 -> rows 512*(t%4) + 128*(t//4)."""
        v_ = []
        for t in range(KT):
            vt = sbt([128, VW], KV_DT, "v", name="vtile")
            r0 = 512 * (t % 4) + 128 * (t // 4)
            nc.gpsimd.dma_start(vt, vout[r0:r0 + 128, :])
            v_.append(vt)
        return v_

    def load_kT_pair(kout, pair):
        """Assemble K^T pair tile [128, 2048] from the AG output, rank-major:
        col block 512*c + 128*s holds global key tile t = c + 4*s (rank c's
        slot s).  Each rank's load is one contiguous [128, 512] DMA."""
        kt_t = sbt([128, S], KV_DT, "kT", name="kTpair")
        for c in range(4):
            nc.gpsimd.dma_start(
                kt_t[:, 512 * c:512 * (c + 1)],
                kout[1024 * c + 128 * pair:1024 * c + 128 * (pair + 1), :])
        return kt_t

    def kt_col(t):
        """Col block of global key tile t in the rank-major kT pair tile."""
        return 512 * (t % 4) + 128 * (t // 4)

    def emit_attn(kout, v_t, qT_t, mask_t, causal):
        """Returns 8 oT pair tiles ([128, 512] bf16)."""
        oT_pairs = []
        kT_all = [load_kT_pair(kout, p) for p in range(PAIRS)]
        for pair in range(PAIRS):
            kT_cur = kT_all[pair]
            pvs = [PS.tile([HD + 1, QL], F32, tag="pv", bufs=2, name="pv")
                   for _ in range(2)]
            for kt_ in range(KT):
                n = QL - 128 * (kt_ // 4) if causal else QL
                q0 = QL - n
                psc = PS.tile([128, 2 * QL], F32, tag="sc", bufs=2, name="sc")
                kc = kt_col(kt_)
                for half in range(2):
                    nc.tensor.matmul(
                        psc[:, half * QL:half * QL + n],
                        kT_cur[half * HD:(half + 1) * HD, kc:kc + 128],
                        qT_t[pair][half * HD:(half + 1) * HD, q0:QL],
                        start=True, stop=True)
                pT = sbt([128, 2 * QL], BF16, "p", name="pT")
                pv_in = psc.rearrange("p (h q) -> p h q", h=2)[:, :, 0:n]
                pT_v = pT.rearrange("p (h q) -> p h q", h=2)
                nc.scalar.activation(pT_v[:, :, 0:n], pv_in, AF.Exp,
                                     scale=0.125)
                if causal:
                    # diagonal tile: first 128 active queries of each half
                    mk = mask_t[:, kt_ * 128:(kt_ + 1) * 128]
                    for half in range(2):
                        nc.vector.tensor_mul(
                            pT[:, half * QL:half * QL + 128],
                            pT[:, half * QL:half * QL + 128], mk)
                for half in range(2):
                    h = pair * 2 + half
                    nc.tensor.matmul(
                        pvs[half][:, q0:QL],
                        v_t[kt_][:, h * (HD + 1):h * (HD + 1) + HD + 1],
                        pT[:, half * QL:half * QL + n],
                        start=(kt_ == 0), stop=(kt_ == KT - 1),
                        skip_group_check=True)
            oT = sbt([128, QL], BF16, "o", name="oT")
            for half in range(2):
                recip = sbt([1, QL], F32, "sm", name="recip")
                nc.vector.reciprocal(recip, pvs[half][HD:HD + 1, :])
                r16 = sbt([1, QL], BF16, "sm", name="r16")
                nc.vector.tensor_copy(r16, recip)
                # row-broadcast recip via PE: [1,64].T @ [1,512] -> [64,512]
                rb = PS.tile([HD, QL], F32, tag="acc", bufs=2, name="rb")
                nc.tensor.matmul(rb, rowb16_t, r16, start=True, stop=True)
                rbs = sbt([HD, QL], F32, "rb", name="rbs")
                nc.vector.tensor_copy(rbs, rb)
                nc.vector.tensor_mul(oT[half * HD:(half + 1) * HD, :],
                                     pvs[half][0:HD, :], rbs)
            oT_pairs.append(oT)
        return oT_pairs

    def emit_out_proj(w_t, in_pairs, bias_key, resid_t):
        """pre[dt] (f32) = W.T @ in_pairs + bias + resid"""
        pre = []
        for m in range(DT):
            acc = PS.tile([128, 512], F32, tag="acc", bufs=2, name="acc")
            for pr in range(PAIRS):
                nc.tensor.matmul(acc, w_t[pr][:, m * 128:(m + 1) * 128],
                                 in_pairs[pr],
                                 start=(pr == 0), stop=(pr == PAIRS - 1))
            t = sbt([128, QL], F32, "res", name="pre")
            nc.vector.scalar_tensor_tensor(t, acc, pap(m, bias_key), resid_t[m],
                                           op0=ALU.add, op1=ALU.add)
            pre.append(t)
        return pre

    def emit_ln(pre_t, g_key, b_key, want_bf16):
        xb, xq_ = [], []
        for dt_ in range(DT):
            t = sbt([128, QL], BF16, "xpre", name="xpre")
            nc.vector.tensor_copy(t, pre_t[dt_])
            xb.append(t)
            t2_ = sbt([128, QL], BF16, "xsq", name="xsq")
            nc.scalar.square(t2_, pre_t[dt_])
            xq_.append(t2_)
        # mean, directly broadcast: [128,128]-of-1/D.T @ xb -> [128, 512]
        meanb = PS.tile([128, QL], F32, tag="acc", bufs=2, name="meanb")
        for dt_ in range(DT):
            nc.tensor.matmul(meanb, meanw_t, xb[dt_], start=(dt_ == 0),
                             stop=(dt_ == DT - 1), skip_group_check=True)
        sxx = PS.tile([1, QL], F32, tag="acc", bufs=2, name="sxx")
        for dt_ in range(DT):
            nc.tensor.matmul(sxx, ones_t, xq_[dt_], start=(dt_ == 0),
                             stop=(dt_ == DT - 1), skip_group_check=True)
        mean = sbt([1, QL], F32, "sm", name="mean")
        nc.vector.tensor_copy(mean, meanb[0:1, :])
        msq = sbt([1, QL], F32, "sm", name="msq")
        nc.vector.tensor_mul(msq, mean, mean)
        var = sbt([1, QL], F32, "sm", name="var")
        nc.vector.scalar_tensor_tensor(var, sxx, 1.0 / D, msq,
                                       op0=ALU.mult, op1=ALU.subtract)
        sd = sbt([1, QL], F32, "sm", name="sd")
        nc.scalar.activation(sd, var, AF.Sqrt, bias=eps_t)
        rstd = sbt([1, QL], F32, "sm", name="rstd")
        nc.vector.reciprocal(rstd, sd)
        # row-broadcast rstd via f32 PE matmul: [1,128].T @ [1,512]
        rstdb = PS.tile([128, QL], F32, tag="acc", bufs=2, name="rstdb")
        nc.tensor.matmul(rstdb, rowb32_t, rstd, start=True, stop=True)
        out32, out16 = [], []
        for dt_ in range(DT):
            t1 = sbt([128, QL], F32, "t1", name="t1")
            nc.vector.tensor_sub(t1, pre_t[dt_], meanb)
            t2_ = sbt([128, QL], F32, "t2", name="t2")
            nc.vector.tensor_mul(t2_, t1, rstdb)
            o32 = sbt([128, QL], F32, "res", name="lnout")
            nc.vector.tensor_scalar(o32, t2_, pap(dt_, g_key), pap(dt_, b_key),
                                    op0=ALU.mult, op1=ALU.add)
            out32.append(o32)
            if want_bf16:
                o16 = sbt([128, QL], BF16, "sb16", name="lnout16")
                nc.vector.tensor_scalar(o16, t2_, pap(dt_, g_key),
                                        pap(dt_, b_key), op0=ALU.mult,
                                        op1=ALU.add)
                out16.append(o16)
        return out32, out16

    # ---------------- the decoder cell ----------------
    import os
    stop_after = os.environ.get("KSTOP", "")

    def _early_out(tiles):
        for dt_ in range(DT):
            nc.sync.dma_start(d["out"][dt_ * 128:(dt_ + 1) * 128, :], tiles[dt_])
        return True

    # K1/V1 pieces first so AG1 triggers as early as possible
    xkp = []
    for dt_ in range(DT):
        t = sbt([128, QL], BF16, "xp", name="xkp")
        nc.sync.dma_start(t, d["xkp"][dt_ * 128:(dt_ + 1) * 128, :])
        xkp.append(t)
    wk1 = load_w("Wk1")
    wv1 = load_w("Wv1")
    emit_kv_piece(wk1, wv1, xkp, "bk1", k1in, v1in, on_act=True)
    emit_ag(k1in, k1out)
    emit_ag(v1in, v1out)

    # overlap the AG window: Q1, K2/V2 pieces (+ their AGs), bulk loads
    x0q = []
    for dt_ in range(DT):
        t = sbt([128, QL], BF16, "sb16", name="x0q")
        nc.sync.dma_start(t, d["x0q"][dt_ * 128:(dt_ + 1) * 128, :])
        x0q.append(t)
    wq1 = load_w("Wq1")
    q1 = emit_q_all(wq1, x0q, "bq1", on_act=True)

    hkp = []
    for dt_ in range(DT):
        t = sbt([128, QL], BF16, "xp", name="hkp")
        nc.sync.dma_start(t, d["hkp"][dt_ * 128:(dt_ + 1) * 128, :])
        hkp.append(t)
    wk2 = load_w("Wk2")
    wv2 = load_w("Wv2")
    emit_kv_piece(wk2, wv2, hkp, "bk2", k2in, v2in, on_act=True)
    emit_ag(k2in, k2out)
    emit_ag(v2in, v2out)

    msk = sbt([128, S], BF16, "m", name="msk")
    nc.sync.dma_start(msk, d["msk"][:, :])
    x0r = []
    for dt_ in range(DT):
        t = sbt([128, QL], F32, "res", name="x0r")
        nc.sync.dma_start(t, d["x0r"][dt_ * 128:(dt_ + 1) * 128, :])
        x0r.append(t)
    wo1 = load_w("Wo1")
    if stop_after == "qkv1":
        _early_out(x0r); return

    v1 = load_v_tiles(v1out)
    o1 = emit_attn(k1out, v1, q1, msk, causal=True)
    if stop_after == "attn1":
        _early_out(x0r); return

    pre1 = emit_out_proj(wo1, o1, "bo1", x0r)
    s1_32, s1_16 = emit_ln(pre1, "g1", "b1", want_bf16=True)
    if stop_after == "ln1":
        _early_out(s1_32); return

    wq2 = load_w("Wq2")
    q2 = emit_q_all(wq2, s1_16, "bq2", on_act=True)
    v2 = load_v_tiles(v2out)
    o2 = emit_attn(k2out, v2, q2, None, causal=False)
    if stop_after == "attn2":
        _early_out(s1_32); return

    wo2 = load_w("Wo2")
    pre2 = emit_out_proj(wo2, o2, "bo2", s1_32)
    s2_32, s2_16 = emit_ln(pre2, "g2", "b2", want_bf16=True)

    wf = load_w("Wf")
    pre3 = emit_out_proj(wf, s2_16, "bf", s2_32)
    s3_32, _ = emit_ln(pre3, "g3", "b3", want_bf16=False)

    for dt_ in range(DT):
        nc.sync.dma_start(d["out"][dt_ * 128:(dt_ + 1) * 128, :], s3_32[dt_])


_CACHE = {}


def build_program():
    if "nc" in _CACHE:
        return _CACHE["nc"]
    nc = bacc.Bacc("TRN2", target_bir_lowering=False, debug=False,
                   num_devices=NC)
    d = {}
    d["xkp"] = nc.dram_tensor("xkp", [D, QL], BF16, kind="ExternalInput")
    d["hkp"] = nc.dram_tensor("hkp", [D, QL], BF16, kind="ExternalInput")
    d["x0q"] = nc.dram_tensor("x0q", [D, QL], BF16, kind="ExternalInput")
    d["x0r"] = nc.dram_tensor("x0r", [D, QL], F32, kind="ExternalInput")
    d["msk"] = nc.dram_tensor("msk", [128, S], BF16, kind="ExternalInput")
    for w in W_NAMES:
        d[w] = nc.dram_tensor(w, [D, D], BF16, kind="ExternalInput")
    d["par"] = nc.dram_tensor("par", [D, NPAR], F32, kind="ExternalInput")
    d["out"] = nc.dram_tensor("out", [D, QL], F32, kind="ExternalOutput")

    from contextlib import ExitStack
    with tile.TileContext(nc) as tc:
        with ExitStack() as ctx:
            _build_body(nc, tc, {k: (v[:] if hasattr(v, "ap") else v)
                                 for k, v in d.items()}, ctx)
    nc.compile()
    _CACHE["nc"] = nc
    return nc


def _key_cols(j):
    """Global key-column indices of core j's shard: tiles j, j+4, j+8, j+12."""
    return np.concatenate([np.arange(128 * (j + 4 * s), 128 * (j + 4 * s) + 128)
                           for s in range(4)])


def make_in_maps(inputs):
    """Build the 8 per-core input dicts from the full problem inputs."""
    bf = ml_dtypes.bfloat16
    S0 = np.asarray(inputs["S0"], np.float32)
    Hh = np.asarray(inputs["H"], np.float32)

    par = np.zeros((D, NPAR), np.float32)
    for key, col in PC.items():
        src = {"bq1": "bq1", "bk1": "bk1", "bo1": "bo1", "g1": "ln1_g",
               "b1": "ln1_b", "bq2": "bq2", "bk2": "bk2", "bo2": "bo2",
               "g2": "ln2_g", "b2": "ln2_b", "bf": "bf", "g3": "ln3_g",
               "b3": "ln3_b"}[key]
        par[:, col] = np.asarray(inputs[src], np.float32)
    # bv folds exactly into bo: a = (o + bv) @ Wo + bo = o @ Wo + (bv @ Wo + bo)
    par[:, PC["bo1"]] += np.asarray(inputs["bv1"], np.float32) @ np.asarray(
        inputs["Wo1"], np.float32)
    par[:, PC["bo2"]] += np.asarray(inputs["bv2"], np.float32) @ np.asarray(
        inputs["Wo2"], np.float32)

    ws = {w: np.ascontiguousarray(np.asarray(inputs[w], np.float32)).astype(bf)
          for w in W_NAMES}

    in_maps = []
    for c in range(NC):
        b, j = c // 4, c % 4
        kc = _key_cols(j)
        qrows = np.arange(QL) * 4 + j          # strided query rows, ascending
        x0t = np.ascontiguousarray(S0[b].T)
        ht = np.ascontiguousarray(Hh[b].T)
        # diagonal masks: tile kt covers keys [128*kt, 128*kt+128) vs
        # queries q = 4*(128*(kt//4) + i') + j
        msk = np.zeros((128, S), np.float32)
        for kt in range(KT):
            i0 = 128 * (kt // 4)
            q = 4 * (i0 + np.arange(128)) + j
            k = 128 * kt + np.arange(128)
            msk[:, 128 * kt:128 * (kt + 1)] = (k[:, None] <= q[None, :])
        m = {
            "xkp": np.ascontiguousarray(x0t[:, kc]).astype(bf),
            "hkp": np.ascontiguousarray(ht[:, kc]).astype(bf),
            "x0q": np.ascontiguousarray(x0t[:, qrows]).astype(bf),
            "x0r": np.ascontiguousarray(x0t[:, qrows]),
            "msk": msk.astype(bf),
            "par": par,
        }
        m.update(ws)
        in_maps.append(m)
    return in_maps


def kernel(**inputs) -> np.ndarray:
    from concourse.bass_utils import run_bass_kernel_spmd
    nc = build_program()
    in_maps = make_in_maps(inputs)
    res = run_bass_kernel_spmd(nc, in_maps, list(range(NC)))
    _CACHE["last_results"] = res
    out = np.zeros((B, S, D), np.float32)
    for c in range(NC):
        b, j = c // 4, c % 4
        qrows = np.arange(QL) * 4 + j
        out[b, qrows, :] = res.results[c]["out"].T
    return out


# revision 41
# speedup vs baseline: 1.3971x; 1.1323x over previous
"""Trainium2 Bass kernel for nn_DecoderCell_59742995087471.

Decoder cell: causal self-attention + add&LN, cross-attention over H + add&LN,
single-Linear FFN + add&LN.  B=2, S=T=2048, D=1024, 16 heads x 64.

Sharding: 8 cores = 2 batch elements x 4 shards.  Within a batch group of 4
cores:
  - queries are assigned STRIDED (core j takes rows j::4 of its batch
    element).  Sorted ascending, the core's 512 queries split into 4
    sub-blocks of 128 whose causal key-range is exactly key tiles
    0..4(g+1)-1 for every core -- so causal skipping is SPMD-uniform and
    attn1 does 62.5% of the full score/PV/exp work.
  - K/V projections are computed sharded: core j projects K/V only for key
    tiles {j, j+4, j+8, j+12} (512 keys) of its batch element, then the
    4-core group AllGathers K^T and V (bf16, ~1MB/rank each) per layer.
    Collectives run on TOPSP/SDMA and overlap compute.

Mask arrives as data only for the 16 diagonal [128k x 128q] tiles and is
applied post-exp on the (otherwise idle) GpSimd/Pool engine.

Layout: activations transposed in SBUF ([d on partitions, rows free]); matmul
operands bf16 (fp32 PSUM accumulate); residual/LN math fp32.  Softmax has no
max-subtraction (scores are O(1) at this data scale) and the denominator
comes from a ones-augmented column in the PV stationary.
"""

import numpy as np
import ml_dtypes

import concourse.bass as bass
import concourse.bacc as bacc
import concourse.mybir as mybir
import concourse.tile as tile

F32 = mybir.dt.float32
BF16 = mybir.dt.bfloat16
FP8 = mybir.dt.float8e4
AF = mybir.ActivationFunctionType
ALU = mybir.AluOpType

FP8_AG = True          # AllGather K/V in fp8e4m3 (half wire bytes)
KV_DT = FP8 if FP8_AG else BF16

B, S, D, H, HD = 2, 2048, 1024, 16, 64
QL = 512          # query rows per core
NC = 8            # cores
GROUPS = [[0, 1, 2, 3], [4, 5, 6, 7]]
DT = D // 128     # 8 d-tiles
KT = S // 128     # 16 key tiles
PAIRS = H // 2    # 8 head pairs
EPS = 1e-5
VW = H * (HD + 1)  # 1040: interleaved V row width (ones-augmented)

W_NAMES = ["Wq1", "Wk1", "Wv1", "Wo1", "Wq2", "Wk2", "Wv2", "Wo2", "Wf"]
PC = {"bq1": 0, "bk1": 1, "bo1": 2, "g1": 3, "b1": 4,
      "bq2": 5, "bk2": 6, "bo2": 7, "g2": 8, "b2": 9,
      "bf": 10, "g3": 11, "b3": 12}
NPAR = 13

BUFS = {
    "xp": 12,    # [128,512] bf16: xkp -> hkp K/V-piece inputs
    "kT": 16,    # [128,2048] kv-dt: K^T pair tiles (both layers resident)
    "v": 2,      # [128,16640] kv-dt: all 16 V tiles in one tile, per layer
    "qT": 9,     # [128,512] bf16 Q^T pairs (8 per layer)
    "sb16": 9,   # [128,512] bf16: x0q, s1_16, s2_16
    "res": 10,   # [128,512] f32 residual stream generations
    "xpre": 2, "xsq": 2,
    "m": 1,      # [128,2048] bf16 diagonal masks (loaded once)
    "p": 4,      # [128,1024] bf16 probs
    "o": 8,      # [128,512] bf16 oT pairs
    "w": 20,     # [128,1024] bf16 weights (rotating)
    "kp": 4,     # [128,512] bf16 K piece outputs awaiting DMA out
    "vp": 2,     # [128,1040] bf16 V piece outputs awaiting DMA out
    "sm": 4,     # [1,512] smalls
    "rb": 2,     # [64,512] f32 broadcast bounce
    "t1": 2, "t2": 2,  # [128,512] f32 LN temps
}


def _build_body(nc, tc, d, ctx):
    pools = {}

    def _pool(tag, bufs, space="SBUF"):
        if tag not in pools:
            pools[tag] = ctx.enter_context(
                tc.tile_pool(name=tag, bufs=bufs, space=space))
        return pools[tag]

    # create every pool up front (before any instruction is emitted)
    for tag, bufs in BUFS.items():
        _pool(tag, bufs)
    for dt_ in range(DT):
        _pool(f"par{dt_}", 1)
    for tag in ("ones", "eps"):
        _pool(tag, 1)
    for tag, bufs in (("acc", 2), ("pv", 2), ("sc", 2)):
        _pool("ps_" + tag, bufs, space="PSUM")
    dram = ctx.enter_context(tc.tile_pool(name="dram", bufs=1, space="DRAM"))

    def sbt(shape, dtype, tag, name=None):
        return _pool(tag, BUFS[tag]).tile(shape, dtype, tag=tag,
                                          name=name or tag)

    class _PS:
        @staticmethod
        def tile(shape, dtype, tag, bufs, name):
            return _pool("ps_" + tag, bufs, space="PSUM").tile(
                shape, dtype, tag=tag, name=name)
    PS = _PS()

    # ---------------- constants / params ----------------
    par_t = []
    for dt_ in range(DT):
        pt = _pool(f"par{dt_}", 1).tile([128, NPAR], F32, name=f"par{dt_}")
        nc.sync.dma_start(pt, d["par"][dt_ * 128:(dt_ + 1) * 128, :])
        par_t.append(pt)
    # bf16 constants: col 0 = ones column (LN sums); cols 1:129 = 1/D
    # (mean-broadcast stationary); cols 129:193 partition0 = ones row
    # (attn recip row-broadcast stationary)
    onesb = _pool("ones", 1).tile([128, 193], BF16, name="onesb")
    nc.vector.memset(onesb, 1.0)
    nc.vector.memset(onesb[:, 1:129], 1.0 / D)
    ones_t = onesb[:, 0:1]
    meanw_t = onesb[:, 1:129]
    rowb16_t = onesb[0:1, 129:193]
    # f32 constants: [1,128] ones row (rstd broadcast stationary) + eps
    onesf = _pool("eps", 1).tile([1, 129], F32, name="onesf")
    nc.vector.memset(onesf[:, 0:128], 1.0)
    nc.vector.memset(onesf[:, 128:129], EPS)
    rowb32_t = onesf[0:1, 0:128]
    eps_t = onesf[0:1, 128:129]

    def pap(dt_, key):
        c = PC[key]
        return par_t[dt_][:, c:c + 1]

    # ---------------- AG dram buffers ----------------
    k1in = dram.tile([D, QL], KV_DT, name="k1in", tag="k1in")
    k1out = dram.tile([4 * D, QL], KV_DT, name="k1out", tag="k1out")
    v1in = dram.tile([QL, VW], KV_DT, name="v1in", tag="v1in")
    v1out = dram.tile([4 * QL, VW], KV_DT, name="v1out", tag="v1out")
    k2in = dram.tile([D, QL], KV_DT, name="k2in", tag="k2in")
    k2out = dram.tile([4 * D, QL], KV_DT, name="k2out", tag="k2out")
    v2in = dram.tile([QL, VW], KV_DT, name="v2in", tag="v2in")
    v2out = dram.tile([4 * QL, VW], KV_DT, name="v2out", tag="v2out")

    # ---------------- building blocks ----------------
    def load_w(name, tag="w"):
        tiles = []
        for dt_ in range(DT):
            t = sbt([128, D], BF16, tag, name=name)
            nc.sync.dma_start(t, d[name][dt_ * 128:(dt_ + 1) * 128, :])
            tiles.append(t)
        return tiles

    def proj_unit(w_t, x_t, out_ap, m, bias_ap, on_act=True):
        """out_ap (bf16 [128,512]) = W[:, m-block].T @ x + bias"""
        acc = PS.tile([128, 512], F32, tag="acc", bufs=2, name="acc")
        for dt_ in range(DT):
            nc.tensor.matmul(acc, w_t[dt_][:, m * 128:(m + 1) * 128],
                             x_t[dt_], start=(dt_ == 0), stop=(dt_ == DT - 1))
        if on_act:
            nc.scalar.activation(out_ap, acc, AF.Identity, bias=bias_ap)
        else:
            nc.vector.tensor_scalar(out_ap, acc, bias_ap, None, op0=ALU.add)
        return acc

    def emit_kv_piece(wk, wv, x_t, bk_key, kin, vin, on_act):
        """Project this core's 512-key shard: K^T piece + interleaved V piece,
        DMA both to the AG input dram tiles."""
        # K^T piece: [1024 dims, 512 keys]
        for m in range(DT):
            t = sbt([128, QL], KV_DT, "kp", name="kpiece")
            proj_unit(wk, x_t, t, m, pap(m, bk_key), on_act=on_act)
            nc.sync.dma_start(kin[m * 128:(m + 1) * 128, :], t)
        # V piece: per local key tile s, interleaved [128, 1040] + ones col
        for s in range(4):
            vt = sbt([128, VW], KV_DT, "vp", name="vpiece")
            nc.vector.memset(
                vt.rearrange("p (h c) -> p h c", h=H)[:, :, HD:HD + 1], 1.0)
            for half in range(2):
                acc = PS.tile([128, 512], F32, tag="acc", bufs=2, name="acc")
                for dt_ in range(DT):
                    nc.tensor.matmul(acc, x_t[dt_][:, s * 128:(s + 1) * 128],
                                     wv[dt_][:, half * 512:(half + 1) * 512],
                                     start=(dt_ == 0), stop=(dt_ == DT - 1))
                vv = vt.rearrange("p (h c) -> p h c", h=H)[
                    :, half * 8:(half + 1) * 8, 0:HD]
                av = acc.rearrange("p (h c) -> p h c", h=8)
                nc.vector.tensor_copy(vv, av)
            nc.sync.dma_start(vin[s * 128:(s + 1) * 128, :], vt)

    def emit_ag(inb, outb):
        nc.gpsimd.collective_compute(
            "AllGather", ALU.bypass, replica_groups=GROUPS,
            ins=[inb.opt()], outs=[outb.opt()])

    def emit_q_all(wq, xq_t, bq_key, on_act=True):
        qT = []
        for pair in range(PAIRS):
            qt = sbt([128, QL], BF16, "qT", name="qT")
            proj_unit(wq, xq_t, qt, pair, pap(pair, bq_key), on_act=on_act)
            qT.append(qt)
        return qT

    def load_v_tiles(vout):
        """Load all 16 gathered V tiles with ONE DMA.  Global key tile t
        lives at AG rows 512*(t%4) + 128*(t//4); the (c s r w -> r s c w)
        rearrange lands tile t at col block t of the combined tile."""
        v_all = sbt([128, KT * VW], KV_DT, "v", name="vall")
        dst = v_all.rearrange("p (s c w) -> p s c w", s=4, c=4)
        for c in range(4):
            nc.gpsimd.dma_start(
                dst[:, :, c, :],
                vout[512 * c:512 * (c + 1), :].rearrange(
                    "(s r) w -> r s w", s=4))
        return [v_all[:, t * VW:(t + 1) * VW] for t in range(KT)]

    def load_kT_pair(kout, pair):
        """Assemble K^T pair tile [128, 2048] from the AG output with one
        DMA, rank-major: col block 512*c + 128*s holds global key tile
        t = c + 4*s (rank c's slot s)."""
        kt_t = sbt([128, S], KV_DT, "kT", name="kTpair")
        nc.gpsimd.dma_start(
            kt_t.rearrange("p (c q) -> p c q", c=4),
            kout[:].rearrange("(c r) q -> r c q", c=4)[
                128 * pair:128 * (pair + 1)])
        return kt_t

    def kt_col(t):
        """Col block of global key tile t in the rank-major kT pair tile."""
        return 512 * (t % 4) + 128 * (t // 4)

    def emit_attn(kT_all, v_t, qT_t, mask_t, causal):
        """Returns 8 oT pair tiles ([128, 512] bf16)."""
        oT_pairs = []
        for pair in range(PAIRS):
            kT_cur = kT_all[pair]
            pvs = [PS.tile([HD + 1, QL], F32, tag="pv", bufs=2, name="pv")
                   for _ in range(2)]
            for kt_ in range(KT):
                n = QL - 128 * (kt_ // 4) if causal else QL
                q0 = QL - n
                psc = PS.tile([128, 2 * QL], F32, tag="sc", bufs=2, name="sc")
                kc = kt_col(kt_)
                for half in range(2):
                    nc.tensor.matmul(
                        psc[:, half * QL:half * QL + n],
                        kT_cur[half * HD:(half + 1) * HD, kc:kc + 128],
                        qT_t[pair][half * HD:(half + 1) * HD, q0:QL],
                        start=True, stop=True)
                pT = sbt([128, 2 * QL], BF16, "p", name="pT")
                pv_in = psc.rearrange("p (h q) -> p h q", h=2)[:, :, 0:n]
                pT_v = pT.rearrange("p (h q) -> p h q", h=2)
                nc.scalar.activation(pT_v[:, :, 0:n], pv_in, AF.Exp,
                                     scale=0.125)
                if causal:
                    # diagonal tile: first 128 active queries of each half
                    mk = mask_t[:, kt_ * 128:(kt_ + 1) * 128]
                    for half in range(2):
                        nc.vector.tensor_mul(
                            pT[:, half * QL:half * QL + 128],
                            pT[:, half * QL:half * QL + 128], mk)
                for half in range(2):
                    h = pair * 2 + half
                    nc.tensor.matmul(
                        pvs[half][:, q0:QL],
                        v_t[kt_][:, h * (HD + 1):h * (HD + 1) + HD + 1],
                        pT[:, half * QL:half * QL + n],
                        start=(kt_ == 0), stop=(kt_ == KT - 1),
                        skip_group_check=True)
            oT = sbt([128, QL], BF16, "o", name="oT")
            for half in range(2):
                recip = sbt([1, QL], F32, "sm", name="recip")
                nc.vector.reciprocal(recip, pvs[half][HD:HD + 1, :])
                r16 = sbt([1, QL], BF16, "sm", name="r16")
                nc.vector.tensor_copy(r16, recip)
                # row-broadcast recip via PE: [1,64].T @ [1,512] -> [64,512]
                rb = PS.tile([HD, QL], F32, tag="acc", bufs=2, name="rb")
                nc.tensor.matmul(rb, rowb16_t, r16, start=True, stop=True)
                rbs = sbt([HD, QL], F32, "rb", name="rbs")
                nc.vector.tensor_copy(rbs, rb)
                nc.vector.tensor_mul(oT[half * HD:(half + 1) * HD, :],
                                     pvs[half][0:HD, :], rbs)
            oT_pairs.append(oT)
        return oT_pairs

    def emit_out_proj(w_t, in_pairs, bias_key, resid_t):
        """pre[dt] (f32) = W.T @ in_pairs + bias + resid"""
        pre = []
        for m in range(DT):
            acc = PS.tile([128, 512], F32, tag="acc", bufs=2, name="acc")
            for pr in range(PAIRS):
                nc.tensor.matmul(acc, w_t[pr][:, m * 128:(m + 1) * 128],
                                 in_pairs[pr],
                                 start=(pr == 0), stop=(pr == PAIRS - 1))
            t = sbt([128, QL], F32, "res", name="pre")
            nc.vector.scalar_tensor_tensor(t, acc, pap(m, bias_key), resid_t[m],
                                           op0=ALU.add, op1=ALU.add)
            pre.append(t)
        return pre

    def emit_ln(pre_t, g_key, b_key, want_bf16):
        xb, xq_ = [], []
        for dt_ in range(DT):
            t = sbt([128, QL], BF16, "xpre", name="xpre")
            nc.scalar.activation(t, pre_t[dt_], AF.Identity)
            xb.append(t)
            t2_ = sbt([128, QL], BF16, "xsq", name="xsq")
            nc.scalar.square(t2_, pre_t[dt_])
            xq_.append(t2_)
        # mean, directly broadcast: [128,128]-of-1/D.T @ xb -> [128, 512]
        meanb = PS.tile([128, QL], F32, tag="acc", bufs=2, name="meanb")
        for dt_ in range(DT):
            nc.tensor.matmul(meanb, meanw_t, xb[dt_], start=(dt_ == 0),
                             stop=(dt_ == DT - 1), skip_group_check=True)
        sxx = PS.tile([1, QL], F32, tag="acc", bufs=2, name="sxx")
        for dt_ in range(DT):
            nc.tensor.matmul(sxx, ones_t, xq_[dt_], start=(dt_ == 0),
                             stop=(dt_ == DT - 1), skip_group_check=True)
        mean = sbt([1, QL], F32, "sm", name="mean")
        nc.vector.tensor_copy(mean, meanb[0:1, :])
        msq = sbt([1, QL], F32, "sm", name="msq")
        nc.vector.tensor_mul(msq, mean, mean)
        var = sbt([1, QL], F32, "sm", name="var")
        nc.vector.scalar_tensor_tensor(var, sxx, 1.0 / D, msq,
                                       op0=ALU.mult, op1=ALU.subtract)
        sd = sbt([1, QL], F32, "sm", name="sd")
        nc.scalar.activation(sd, var, AF.Sqrt, bias=eps_t)
        rstd = sbt([1, QL], F32, "sm", name="rstd")
        nc.vector.reciprocal(rstd, sd)
        # row-broadcast rstd via f32 PE matmul: [1,128].T @ [1,512]
        rstdb = PS.tile([128, QL], F32, tag="acc", bufs=2, name="rstdb")
        nc.tensor.matmul(rstdb, rowb32_t, rstd, start=True, stop=True)
        out32, out16 = [], []
        for dt_ in range(DT):
            t1 = sbt([128, QL], F32, "t1", name="t1")
            nc.vector.tensor_sub(t1, pre_t[dt_], meanb)
            t2_ = sbt([128, QL], F32, "t2", name="t2")
            nc.vector.tensor_mul(t2_, t1, rstdb)
            o32 = sbt([128, QL], F32, "res", name="lnout")
            nc.vector.tensor_scalar(o32, t2_, pap(dt_, g_key), pap(dt_, b_key),
                                    op0=ALU.mult, op1=ALU.add)
            out32.append(o32)
            if want_bf16:
                o16 = sbt([128, QL], BF16, "sb16", name="lnout16")
                nc.vector.tensor_scalar(o16, t2_, pap(dt_, g_key),
                                        pap(dt_, b_key), op0=ALU.mult,
                                        op1=ALU.add)
                out16.append(o16)
        return out32, out16

    # ---------------- the decoder cell ----------------
    import os
    stop_after = os.environ.get("KSTOP", "")

    def _early_out(tiles):
        for dt_ in range(DT):
            nc.sync.dma_start(d["out"][dt_ * 128:(dt_ + 1) * 128, :], tiles[dt_])
        return True

    # K1/V1 pieces first so AG1 triggers as early as possible
    xkp = []
    for dt_ in range(DT):
        t = sbt([128, QL], BF16, "xp", name="xkp")
        nc.sync.dma_start(t, d["xkp"][dt_ * 128:(dt_ + 1) * 128, :])
        xkp.append(t)
    wk1 = load_w("Wk1")
    wv1 = load_w("Wv1")
    emit_kv_piece(wk1, wv1, xkp, "bk1", k1in, v1in, on_act=True)
    emit_ag(k1in, k1out)
    emit_ag(v1in, v1out)

    # overlap the AG window: Q1, K2/V2 pieces (+ their AGs), bulk loads
    x0q = []
    for dt_ in range(DT):
        t = sbt([128, QL], BF16, "sb16", name="x0q")
        nc.sync.dma_start(t, d["x0q"][dt_ * 128:(dt_ + 1) * 128, :])
        x0q.append(t)
    wq1 = load_w("Wq1")
    q1 = emit_q_all(wq1, x0q, "bq1", on_act=True)

    hkp = []
    for dt_ in range(DT):
        t = sbt([128, QL], BF16, "xp", name="hkp")
        nc.sync.dma_start(t, d["hkp"][dt_ * 128:(dt_ + 1) * 128, :])
        hkp.append(t)
    wk2 = load_w("Wk2")
    wv2 = load_w("Wv2")
    emit_kv_piece(wk2, wv2, hkp, "bk2", k2in, v2in, on_act=True)
    emit_ag(k2in, k2out)
    emit_ag(v2in, v2out)

    msk = sbt([128, S], BF16, "m", name="msk")
    nc.sync.dma_start(msk, d["msk"][:, :])
    x0r = []
    for dt_ in range(DT):
        t = sbt([128, QL], F32, "res", name="x0r")
        nc.sync.dma_start(t, d["x0r"][dt_ * 128:(dt_ + 1) * 128, :])
        x0r.append(t)
    wo1 = load_w("Wo1")
    if stop_after == "qkv1":
        _early_out(x0r); return

    v1 = load_v_tiles(v1out)
    kT1 = [load_kT_pair(k1out, p) for p in range(PAIRS)]
    v2 = load_v_tiles(v2out)
    kT2 = [load_kT_pair(k2out, p) for p in range(PAIRS)]
    o1 = emit_attn(kT1, v1, q1, msk, causal=True)
    if stop_after == "attn1":
        _early_out(x0r); return

    pre1 = emit_out_proj(wo1, o1, "bo1", x0r)
    s1_32, s1_16 = emit_ln(pre1, "g1", "b1", want_bf16=True)
    if stop_after == "ln1":
        _early_out(s1_32); return

    wq2 = load_w("Wq2")
    q2 = emit_q_all(wq2, s1_16, "bq2", on_act=True)
    o2 = emit_attn(kT2, v2, q2, None, causal=False)
    if stop_after == "attn2":
        _early_out(s1_32); return

    wo2 = load_w("Wo2")
    pre2 = emit_out_proj(wo2, o2, "bo2", s1_32)
    s2_32, s2_16 = emit_ln(pre2, "g2", "b2", want_bf16=True)

    wf = load_w("Wf")
    pre3 = emit_out_proj(wf, s2_16, "bf", s2_32)
    s3_32, _ = emit_ln(pre3, "g3", "b3", want_bf16=False)

    for dt_ in range(DT):
        nc.sync.dma_start(d["out"][dt_ * 128:(dt_ + 1) * 128, :], s3_32[dt_])


_CACHE = {}


def build_program():
    if "nc" in _CACHE:
        return _CACHE["nc"]
    nc = bacc.Bacc("TRN2", target_bir_lowering=False, debug=False,
                   num_devices=NC)
    d = {}
    d["xkp"] = nc.dram_tensor("xkp", [D, QL], BF16, kind="ExternalInput")
    d["hkp"] = nc.dram_tensor("hkp", [D, QL], BF16, kind="ExternalInput")
    d["x0q"] = nc.dram_tensor("x0q", [D, QL], BF16, kind="ExternalInput")
    d["x0r"] = nc.dram_tensor("x0r", [D, QL], F32, kind="ExternalInput")
    d["msk"] = nc.dram_tensor("msk", [128, S], BF16, kind="ExternalInput")
    for w in W_NAMES:
        d[w] = nc.dram_tensor(w, [D, D], BF16, kind="ExternalInput")
    d["par"] = nc.dram_tensor("par", [D, NPAR], F32, kind="ExternalInput")
    d["out"] = nc.dram_tensor("out", [D, QL], F32, kind="ExternalOutput")

    from contextlib import ExitStack
    with tile.TileContext(nc) as tc:
        with ExitStack() as ctx:
            _build_body(nc, tc, {k: (v[:] if hasattr(v, "ap") else v)
                                 for k, v in d.items()}, ctx)
    nc.compile()
    _CACHE["nc"] = nc
    return nc


def _key_cols(j):
    """Global key-column indices of core j's shard: tiles j, j+4, j+8, j+12."""
    return np.concatenate([np.arange(128 * (j + 4 * s), 128 * (j + 4 * s) + 128)
                           for s in range(4)])


def make_in_maps(inputs):
    """Build the 8 per-core input dicts from the full problem inputs."""
    bf = ml_dtypes.bfloat16
    S0 = np.asarray(inputs["S0"], np.float32)
    Hh = np.asarray(inputs["H"], np.float32)

    par = np.zeros((D, NPAR), np.float32)
    for key, col in PC.items():
        src = {"bq1": "bq1", "bk1": "bk1", "bo1": "bo1", "g1": "ln1_g",
               "b1": "ln1_b", "bq2": "bq2", "bk2": "bk2", "bo2": "bo2",
               "g2": "ln2_g", "b2": "ln2_b", "bf": "bf", "g3": "ln3_g",
               "b3": "ln3_b"}[key]
        par[:, col] = np.asarray(inputs[src], np.float32)
    # bv folds exactly into bo: a = (o + bv) @ Wo + bo = o @ Wo + (bv @ Wo + bo)
    par[:, PC["bo1"]] += np.asarray(inputs["bv1"], np.float32) @ np.asarray(
        inputs["Wo1"], np.float32)
    par[:, PC["bo2"]] += np.asarray(inputs["bv2"], np.float32) @ np.asarray(
        inputs["Wo2"], np.float32)

    ws = {w: np.ascontiguousarray(np.asarray(inputs[w], np.float32)).astype(bf)
          for w in W_NAMES}

    in_maps = []
    for c in range(NC):
        b, j = c // 4, c % 4
        kc = _key_cols(j)
        qrows = np.arange(QL) * 4 + j          # strided query rows, ascending
        x0t = np.ascontiguousarray(S0[b].T)
        ht = np.ascontiguousarray(Hh[b].T)
        # diagonal masks: tile kt covers keys [128*kt, 128*kt+128) vs
        # queries q = 4*(128*(kt//4) + i') + j
        msk = np.zeros((128, S), np.float32)
        for kt in range(KT):
            i0 = 128 * (kt // 4)
            q = 4 * (i0 + np.arange(128)) + j
            k = 128 * kt + np.arange(128)
            msk[:, 128 * kt:128 * (kt + 1)] = (k[:, None] <= q[None, :])
        m = {
            "xkp": np.ascontiguousarray(x0t[:, kc]).astype(bf),
            "hkp": np.ascontiguousarray(ht[:, kc]).astype(bf),
            "x0q": np.ascontiguousarray(x0t[:, qrows]).astype(bf),
            "x0r": np.ascontiguousarray(x0t[:, qrows]),
            "msk": msk.astype(bf),
            "par": par,
        }
        m.update(ws)
        in_maps.append(m)
    return in_maps


def kernel(**inputs) -> np.ndarray:
    from concourse.bass_utils import run_bass_kernel_spmd
    nc = build_program()
    in_maps = make_in_maps(inputs)
    res = run_bass_kernel_spmd(nc, in_maps, list(range(NC)))
    _CACHE["last_results"] = res
    out = np.zeros((B, S, D), np.float32)
    for c in range(NC):
        b, j = c // 4, c % 4
        qrows = np.arange(QL) * 4 + j
        out[b, qrows, :] = res.results[c]["out"].T
    return out
